# revision 51
# baseline (speedup 1.0000x reference)
"""CaMoE block (LayerNorm -> per-expert squared-ReLU FFN with top-1 routing,
confidence-scaled combine, residual) on 8 Trainium2 NeuronCores.

Strategy (token-parallel with expert-grouped tiles):
  * Host: stable-sort tokens by winning expert, pad each expert group to a
    multiple of 128*8 so every core receives the SAME number of 128-token
    tiles per expert. This makes the SPMD program identical across cores
    while every 128-token tile has a single expert.
  * Device (per core): for each 128-token tile: LayerNorm (token-major),
    confidence sigmoid(h.wc+bc) and straight-through scale c/(c+1e-6);
    transpose h via the PE; then stream the expert's W1/W2 in H-chunks and
    run  kT = relu(W1^T h^T)^2  (PE + DVE) and  y += kT^T W2chunk  (PE),
    finally  out = y*scale + x  (DVE) and DMA out.
  * Host: scatter rows back to their original token positions.

gamma/beta of the LayerNorm are folded into W1/wc on the host (plus an
additive H-bias when beta != 0), so the device computes the pre-affine LN.
All matmuls run in bf16 with fp32 PSUM accumulation.
"""

import math
import os
from contextlib import ExitStack

import numpy as np

import concourse.bass as bass
import concourse.mybir as mybir
import concourse.tile as tile
from concourse.bass_utils import run_bass_kernel_spmd
from concourse.masks import make_identity
from concourse.tile import TileContext, ScopedClock

AF = mybir.ActivationFunctionType
OP = mybir.AluOpType
BF16 = mybir.dt.bfloat16
F32 = mybir.dt.float32
NP_BF16 = mybir.dt.np(BF16)

NCORES = 8
TILE = 128
HCHUNK = 512
LN_EPS = 1e-5

# ---------------------------------------------------------------------------
# Workarounds for the walrus build in this environment: it encodes at most
# ONE semaphore wait per instruction and cannot split multi-wait
# instructions itself ("Too many sync wait commands"). We (a) emit the
# TileContext tail-drain waits one-per-NoOp and (b) post-process the whole
# program to hoist excess waits onto same-engine NoOps.
# ---------------------------------------------------------------------------


def _patched_drain_and_barrier(self, tick_clock, wait_clock):
    probe = self.nc.sync.nop(nofuse=True)
    wait_clock.add_sem_waits(probe.ins, ScopedClock({None: tick_clock.global_clock}))
    si = probe.ins.sync_info
    waits = list(si.on_wait) if si is not None and si.on_wait else []
    if len(waits) > 1:
        probe.ins.sync_info = mybir.SyncInfo(on_wait=[waits[0]], on_update=[])
        for w in waits[1:]:
            n = self.nc.sync.nop(nofuse=True)
            n.ins.sync_info = mybir.SyncInfo(on_wait=[w], on_update=[])
    self.nc.sync.drain()
    self.nc.all_engine_barrier()
    assert self.sems is not None
    popped = self.nc._tile_sem_poison_stack.pop()
    assert popped is self._sem_poison
    self.nc.clear_and_free_semaphores(list(self.sems.allocated().values()))
    self.nc.all_engine_barrier()


TileContext._drain_and_barrier = _patched_drain_and_barrier


def _split_excess_waits(nc, max_waits: int = 1):
    for fn in nc.m.functions:
        for bb in fn.blocks:
            insts = list(bb.instructions)
            out = []
            changed = False
            for inst in insts:
                si = inst.sync_info
                waits = list(si.on_wait) if si is not None and si.on_wait else []
                if len(waits) > max_waits:
                    extra = waits[:-max_waits]
                    keep = waits[-max_waits:]
                    for j, w in enumerate(extra):
                        nop = mybir.InstNoOp(
                            name=f"{inst.name}-wsplit{j}", ins=[], outs=[]
                        )
                        nop.engine = inst.engine
                        nop.sync_info = mybir.SyncInfo(on_wait=[w], on_update=[])
                        out.append(nop)
                    inst.sync_info = mybir.SyncInfo(
                        on_wait=keep,
                        on_update=list(si.on_update) if si.on_update else [],
                    )
                    changed = True
                out.append(inst)
            if changed:
                bb.instructions = out


# ---------------------------------------------------------------------------
# Device program
# ---------------------------------------------------------------------------


def _build_program(C, H, M, S, passes, zero_bias):
    """Emit the SPMD Bass program. `passes` is a list of
    (slot, tile_offset, n_tiles<=2); every core runs the same program on its
    own data."""
    NKC = C // TILE          # K-tiles over C (8)
    NMH = HCHUNK // TILE     # M-tiles per H-chunk (4)
    NHC = H // HCHUNK        # H-chunks (8)
    NC2 = C // 512           # output column chunks (2)
    HN = H // TILE           # bias columns (32)

    WCOLS = NKC * HCHUNK + NMH * C  # w1-part then w2-part, tile-contiguous

    nc = bass.Bass("TRN2", target_bir_lowering=False, debug=False)
    xc = nc.dram_tensor("xc", [M, C], F32, kind="ExternalInput").ap()
    wr = nc.dram_tensor("wr", [S, NHC, TILE, WCOLS], BF16, kind="ExternalInput").ap()
    wcb = nc.dram_tensor("wcb", [S, TILE, C], BF16, kind="ExternalInput").ap()
    bcs = nc.dram_tensor("bcs", [S, TILE, 1], F32, kind="ExternalInput").ap()
    if not zero_bias:
        b1b = nc.dram_tensor("b1b", [S, TILE, HN], F32, kind="ExternalInput").ap()
    yc = nc.dram_tensor("yc", [M, C], F32, kind="ExternalOutput").ap()

    with TileContext(nc) as tc, ExitStack() as ctx:
        cpool = ctx.enter_context(tc.tile_pool(name="const", bufs=1))
        ident = cpool.tile([TILE, TILE], BF16, tag="ident")
        make_identity(nc, ident[:])
        epsc = cpool.tile([TILE, 1], F32, tag="eps")
        nc.gpsimd.memset(epsc[:], LN_EPS)

        # weights stay RESIDENT for a whole slot (8 chunks x 2MB); the pool
        # rotation naturally overlaps the next slot's loads with the current
        # slot's last-pass reads
        wpool = ctx.enter_context(tc.tile_pool(name="w", bufs=8))
        spool = ctx.enter_context(tc.tile_pool(name="slot", bufs=2))
        xpool = ctx.enter_context(tc.tile_pool(name="x", bufs=4))
        hpool = ctx.enter_context(tc.tile_pool(name="h", bufs=4))
        prpool = ctx.enter_context(tc.tile_pool(name="pr", bufs=2))
        htpool = ctx.enter_context(tc.tile_pool(name="ht", bufs=2))
        kpool = ctx.enter_context(tc.tile_pool(name="kt", bufs=4))
        opool = ctx.enter_context(tc.tile_pool(name="o", bufs=3))
        stpool = ctx.enter_context(tc.tile_pool(name="st", bufs=8))
        sqpool = ctx.enter_context(tc.tile_pool(name="sq", bufs=1))
        pps = ctx.enter_context(tc.tile_pool(name="pk", bufs=2, space="PSUM"))
        ppy = ctx.enter_context(tc.tile_pool(name="py", bufs=4, space="PSUM"))
        ppt = ctx.enter_context(tc.tile_pool(name="ptr", bufs=2, space="PSUM"))

        slot_consts = {}
        w_chunks = {}

        def get_slot_consts(si):
            # NOTE: spool bufs must cover the number of distinct slots alive
            # at once (current + next pass's). Entries are invalidated by the
            # pool's slot reuse; with bufs=2 and passes grouped by slot this
            # holds.
            if si not in slot_consts:
                wcb_sb = spool.tile([TILE, C], BF16, tag="wcb", name=f"wcb{si}")
                nc.sync.dma_start(wcb_sb[:], wcb[si])
                bct = spool.tile([TILE, 1], F32, tag="bc", name=f"bc{si}")
                nc.sync.dma_start(bct[:], bcs[si])
                b1_sb = None
                if not zero_bias:
                    b1_sb = spool.tile([TILE, HN], F32, tag="b1", name=f"b1{si}")
                    nc.sync.dma_start(b1_sb[:], b1b[si])
                slot_consts[si] = (wcb_sb, bct, b1_sb)
            return slot_consts[si]

        def emit_stage_a(pass_idx):
            si, tile_off, nt = passes[pass_idx]
            ntok = TILE * nt
            wcb_sb, bct, _ = get_slot_consts(si)
            x_t = []
            s_t = []
            hT = htpool.tile([TILE, NKC, ntok], BF16, tag="hT",
                             name=f"hT{pass_idx}")
            # ops are interleaved across the pass's tiles so the serial
            # dep-chain latency (with a sem hop per tiny op) is paid once,
            # not once per tile
            st = lambda tag: [stpool.tile([TILE, 1], F32, tag=tag, name=f"{tag}{pass_idx}_{t}") for t in range(nt)]
            for t in range(nt):
                row0 = (tile_off + t) * TILE
                xt = xpool.tile([TILE, C], F32, tag="x", name=f"x{pass_idx}_{t}")
                x_t.append(xt)
                nc.sync.dma_start(xt[:], xc[row0 : row0 + TILE, :])
            nsum, negmu, ssq, std, rs, nmrs = (
                st("nsum"), st("negmu"), st("ssq"), st("std"), st("rs"),
                st("nmrs"),
            )
            cdot, conf, cpe, rc = st("cdot"), st("conf"), st("cpe"), st("rc")
            hts = []
            for t in range(nt):
                nc.vector.reduce_sum(
                    nsum[t][:], x_t[t][:], axis=mybir.AxisListType.X,
                    negate=True,
                )
            for t in range(nt):
                nc.scalar.mul(negmu[t][:], nsum[t][:], 1.0 / C)
            if zero_bias:
                # Fast path: transpose xc = x - mu (available right after the
                # mean); 1/std commutes through relu^2 and folds into the
                # stage-C per-token scalar:
                #   relu(rs*(xc@W1))^2 @ W2 = rs^2 * (relu(xc@W1)^2 @ W2)
                for t in range(nt):
                    ht_ = hpool.tile([TILE, C], BF16, tag="h",
                                     name=f"h{pass_idx}_{t}")
                    hts.append(ht_)
                    nc.scalar.activation(
                        ht_[:], x_t[t][:], AF.Identity, bias=negmu[t][:],
                        scale=1.0,
                    )
                for t in range(nt):
                    sq = sqpool.tile([TILE, C], F32, tag="sq")
                    nc.scalar.activation(
                        sq[:], x_t[t][:], AF.Square, bias=negmu[t][:],
                        scale=1.0, accum_out=ssq[t][:],
                    )
                for t in range(nt):
                    nc.scalar.activation(
                        std[t][:], ssq[t][:], AF.Sqrt, bias=epsc[:],
                        scale=1.0 / C,
                    )
                for t in range(nt):
                    nc.vector.reciprocal(rs[t][:], std[t][:])
                for t in range(nt):
                    prod = prpool.tile([TILE, C], BF16, tag="prod")
                    nc.vector.scalar_tensor_tensor(
                        prod[:], hts[t][:], 1.0, wcb_sb[:], op0=OP.mult,
                        op1=OP.mult, accum_out=cdot[t][:],
                    )
                for t in range(nt):
                    nc.scalar.activation(
                        conf[t][:], cdot[t][:], AF.Sigmoid, bias=bct[:],
                        scale=rs[t][:],
                    )
                for t in range(nt):
                    nc.vector.tensor_scalar_add(cpe[t][:], conf[t][:], 1e-6)
                for t in range(nt):
                    nc.vector.reciprocal(rc[t][:], cpe[t][:])
                rs2 = st("rs2")
                sc0 = st("sc0")
                for t in range(nt):
                    nc.vector.tensor_mul(rs2[t][:], rs[t][:], rs[t][:])
                for t in range(nt):
                    nc.vector.tensor_mul(sc0[t][:], conf[t][:], rc[t][:])
                for t in range(nt):
                    sc = stpool.tile([TILE, 1], F32, tag="sc",
                                     name=f"sc{pass_idx}_{t}")
                    nc.vector.tensor_mul(sc[:], sc0[t][:], rs2[t][:])
                    s_t.append(sc)
                return x_t, s_t, hT, hts
            for t in range(nt):
                sq = sqpool.tile([TILE, C], F32, tag="sq")
                nc.scalar.activation(
                    sq[:], x_t[t][:], AF.Square, bias=negmu[t][:], scale=1.0,
                    accum_out=ssq[t][:],
                )
            for t in range(nt):
                nc.scalar.activation(
                    std[t][:], ssq[t][:], AF.Sqrt, bias=epsc[:], scale=1.0 / C
                )
            for t in range(nt):
                nc.vector.reciprocal(rs[t][:], std[t][:])
            for t in range(nt):
                nc.vector.tensor_mul(nmrs[t][:], negmu[t][:], rs[t][:])
            for t in range(nt):
                ht_ = hpool.tile([TILE, C], BF16, tag="h", name=f"h{pass_idx}_{t}")
                hts.append(ht_)
                nc.scalar.activation(
                    ht_[:], x_t[t][:], AF.Identity, bias=nmrs[t][:],
                    scale=rs[t][:],
                )
            for t in range(nt):
                prod = prpool.tile([TILE, C], BF16, tag="prod")
                nc.vector.scalar_tensor_tensor(
                    prod[:], hts[t][:], 1.0, wcb_sb[:], op0=OP.mult,
                    op1=OP.mult, accum_out=cdot[t][:],
                )
            for t in range(nt):
                nc.scalar.activation(
                    conf[t][:], cdot[t][:], AF.Sigmoid, bias=bct[:], scale=1.0
                )
            for t in range(nt):
                nc.vector.tensor_scalar_add(cpe[t][:], conf[t][:], 1e-6)
            for t in range(nt):
                nc.vector.reciprocal(rc[t][:], cpe[t][:])
            for t in range(nt):
                sc = stpool.tile([TILE, 1], F32, tag="sc", name=f"sc{pass_idx}_{t}")
                nc.vector.tensor_mul(sc[:], conf[t][:], rc[t][:])
                s_t.append(sc)
            return x_t, s_t, hT, hts

        def emit_transposes(pass_idx):
            # phase 2: emitted well after phase 1 so the DVE copies never
            # head-of-line block the relu^2 stream waiting on the LN chain
            _, _, nt = passes[pass_idx]
            x_t, s_t, hT, hts = stage_a[pass_idx]
            for t in range(nt):
                for kc in range(NKC):
                    pt = ppt.tile([TILE, TILE], BF16, tag="ptr")
                    nc.tensor.transpose(
                        pt[:], hts[t][:, kc * TILE : (kc + 1) * TILE], ident[:]
                    )
                    nc.vector.tensor_copy(
                        hT[:, kc, t * TILE : (t + 1) * TILE], pt[:]
                    )

        stage_a = {}
        transposed = set()

        def get_stage_a(pass_idx):
            if pass_idx < len(passes) and pass_idx not in stage_a:
                stage_a[pass_idx] = emit_stage_a(pass_idx)
            return stage_a.get(pass_idx)

        def get_transposes(pass_idx):
            if pass_idx < len(passes) and pass_idx not in transposed:
                get_stage_a(pass_idx)
                transposed.add(pass_idx)
                emit_transposes(pass_idx)

        pending_stage_c = []
        for pass_idx, (si, tile_off, nt) in enumerate(passes):
            ntok = TILE * nt
            get_transposes(pass_idx)
            x_t, s_t, hT, _ = stage_a[pass_idx]
            _, _, b1_sb = get_slot_consts(si)

            ys = [
                ppy.tile([TILE, 512], F32, tag="py", name=f"ys{i}")
                for i in range(nt * NC2)
            ]

            def emit_mm2(hc, mh, kt, wt):
                w2base = NKC * HCHUNK + mh * C
                for t in range(nt):
                    for ncx in range(NC2):
                        nc.tensor.matmul(
                            ys[t * NC2 + ncx][:],
                            kt[:, t * TILE : (t + 1) * TILE],
                            wt[:, w2base + ncx * 512 : w2base + (ncx + 1) * 512],
                            start=(hc == 0 and mh == 0),
                            stop=(hc == NHC - 1 and mh == NMH - 1),
                        )

            # mm2(g) depends on the ACT/DVE relu^2 of mm1(g)'s psum, and the
            # first mm2 of a pass also waits on the previous pass's combine
            # (ys release); the PE is in-order, so emit mm2(g) two mm1
            # groups later to hide both latencies.
            pending = []  # [(hc, mh, kt, wt), ...]
            for hc in range(NHC):
                if (si, hc) not in w_chunks:
                    wt = wpool.tile([TILE, WCOLS], BF16, tag="w",
                                    name=f"w{si}_{hc}")
                    nc.sync.dma_start(wt[:], wr[si, hc])
                    w_chunks[(si, hc)] = wt
                wt = w_chunks[(si, hc)]
                if hc == 1:
                    # next pass's LayerNorm/conf: in-order ACT/DVE streams
                    # reach them mid-pass instead of after all relu ops
                    get_stage_a(pass_idx + 1)
                if hc == 4:
                    get_transposes(pass_idx + 1)
                for mh in range(NMH):
                    pk = pps.tile([TILE, ntok], F32, tag="pk")
                    for kc in range(NKC):
                        nc.tensor.matmul(
                            pk[:],
                            wt[:, kc * HCHUNK + mh * TILE : kc * HCHUNK + (mh + 1) * TILE],
                            hT[:, kc, :],
                            start=(kc == 0),
                            stop=(kc == NKC - 1),
                        )
                    kt = kpool.tile([TILE, ntok], BF16, tag="kt")
                    kr = kpool.tile([TILE, ntok], BF16, tag="kr")
                    if zero_bias:
                        # alternate relu between DVE and ACT to halve each
                        # engine's share of the mm2-feeding path
                        if mh % 2 == 0:
                            nc.vector.tensor_scalar_max(kr[:], pk[:], 0.0)
                        else:
                            nc.scalar.activation(
                                kr[:], pk[:], AF.Relu, bias=0.0, scale=1.0
                            )
                    else:
                        col = hc * NMH + mh
                        nc.scalar.activation(
                            kr[:], pk[:], AF.Relu,
                            bias=b1_sb[:, col : col + 1], scale=1.0,
                        )
                    nc.vector.tensor_mul(kt[:], kr[:], kr[:])
                    pending.append((hc, mh, kt, wt))
                    if hc == 0 and pending_stage_c:
                        # previous pass's combine, one tile per mm1 group:
                        # frees that pass's ys psum banks incrementally right
                        # behind this pass's first relus
                        pending_stage_c.pop(0)()
                    if len(pending) > 2:
                        emit_mm2(*pending.pop(0))
            while pending:
                emit_mm2(*pending.pop(0))

            def stage_c_tile(t, ys=ys, x_t=x_t, s_t=s_t, tile_off=tile_off):
                row0 = (tile_off + t) * TILE
                ot = opool.tile([TILE, C], F32, tag="o")
                for ncx in range(NC2):
                    nc.vector.scalar_tensor_tensor(
                        ot[:, ncx * 512 : (ncx + 1) * 512],
                        ys[t * NC2 + ncx][:],
                        s_t[t][:],
                        x_t[t][:, ncx * 512 : (ncx + 1) * 512],
                        op0=OP.mult,
                        op1=OP.add,
                    )
                nc.sync.dma_start(yc[row0 : row0 + TILE, :], ot[:])

            while pending_stage_c:
                pending_stage_c.pop(0)()
            pending_stage_c.extend(
                (lambda t=t: stage_c_tile(t)) for t in range(nt)
            )
        while pending_stage_c:
            pending_stage_c.pop(0)()

    _split_excess_waits(nc, 1)
    return nc


# ---------------------------------------------------------------------------
# Host-side dispatch
# ---------------------------------------------------------------------------


def _partitions(total, max_part, max_len):
    if total == 0:
        yield ()
        return
    if max_len == 0:
        return
    for first in range(min(total, max_part), 0, -1):
        for rest in _partitions(total - first, first, max_len - 1):
            yield (first,) + rest


def _try_pack(tiles, Tvec):
    """Greedy: assign each expert (desc) slot instances (8 per slot type).
    Returns assign list aligned with `tiles` order, or None."""
    avail = [list(range(NCORES)) for _ in Tvec]
    order_i = sorted(range(len(tiles)), key=lambda i: -tiles[i])
    assign = [None] * len(tiles)
    sizes = sorted(range(len(Tvec)), key=lambda j: -Tvec[j])
    for i in order_i:
        rem = tiles[i]
        inst = []
        while rem > 0:
            # largest slot type with size <= rem, else smallest type >= rem
            pick = None
            for j in sizes:
                if avail[j] and Tvec[j] <= rem:
                    pick = j
                    break
            if pick is None:
                for j in reversed(sizes):
                    if avail[j]:
                        pick = j
                        break
            if pick is None:
                return None
            c = avail[pick].pop(0)
            inst.append((pick, c))
            rem -= Tvec[pick]
        assign[i] = inst
    return assign


def _pack_slots(tiles):
    """Choose per-core slot sizes Tvec (identical structure on all cores)
    and an (expert -> slot instances) assignment minimizing per-core tiles."""
    total = sum(tiles)
    pmin = int(math.ceil(total / NCORES))
    for P in range(pmin, pmin + 4):
        cands = sorted(_partitions(P, P, 6), key=len)
        for Tvec in cands:
            a = _try_pack(tiles, list(Tvec))
            if a is not None:
                return list(Tvec), a
    # fallback: one dedicated slot per expert on every core
    Tvec = [int(math.ceil(t / NCORES)) for t in tiles]
    assign = [[(j, c) for c in range(NCORES)] for j in range(len(tiles))]
    return Tvec, assign


def _prepare(x, winners, gamma, beta, w1, w2, wc, bc):
    x = np.ascontiguousarray(np.asarray(x, dtype=np.float32))
    winners = np.asarray(winners).reshape(-1).astype(np.int64)
    gamma = np.asarray(gamma, dtype=np.float32)
    beta = np.asarray(beta, dtype=np.float32)
    w1 = np.asarray(w1, dtype=np.float32)
    w2 = np.asarray(w2, dtype=np.float32)
    wc = np.asarray(wc, dtype=np.float32)
    bc = np.asarray(bc, dtype=np.float32)

    B, T, C = x.shape
    E, _, H = w1.shape
    N = B * T
    xf = x.reshape(N, C)

    order = np.argsort(winners, kind="stable")
    counts = np.bincount(winners, minlength=E)

    present = [e for e in range(E) if counts[e] > 0]
    tiles_e = {e: int(math.ceil(counts[e] / TILE)) for e in present}

    Tvec, assign = _pack_slots([tiles_e[e] for e in present])
    # assign[i] = list of (slot_j, core_c) instances for present[i]
    S = len(Tvec)

    # slot_expert[c][j]: which expert's weights core c streams in slot j
    # (dummy instances reuse expert present[0]'s weights; their tokens are 0)
    slot_expert = [[present[0]] * S for _ in range(NCORES)]
    # token index list per (core, slot): length Tvec[j]*TILE, -1 = padding
    slot_idx = [
        [np.full(Tvec[j] * TILE, -1, dtype=np.int64) for j in range(S)]
        for c in range(NCORES)
    ]
    pos = 0
    for i, e in enumerate(present):
        n_e = int(counts[e])
        toks = order[pos : pos + n_e]
        pos += n_e
        filled = 0
        for (j, c) in assign[i]:
            slot_expert[c][j] = e
            cap = Tvec[j] * TILE
            take = min(cap, n_e - filled)
            if take > 0:
                slot_idx[c][j][:take] = toks[filled : filled + take]
                filled += take
        assert filled == n_e

    per_core_idx = [np.concatenate(slot_idx[c]) for c in range(NCORES)]
    M = per_core_idx[0].size

    passes = []
    tile_off = 0
    for j in range(S):
        k = 0
        while k < Tvec[j]:
            nt = min(2, Tvec[j] - k)
            passes.append((j, tile_off + k, nt))
            k += nt
        tile_off += Tvec[j]

    # fold gamma/beta; build per-EXPERT folded tensors once, then per-core
    # stacks indexed by that core's slot->expert table
    NKC = C // TILE
    NMH = HCHUNK // TILE
    NHC = H // HCHUNK
    zero_bias = bool(np.all(beta == 0.0))
    wrearr_e = {}
    wcb_e = {}
    bc_e = {}
    b1b_e = {}
    for e in present:
        w1f = (w1[e] * gamma[:, None]).astype(NP_BF16)
        w2f = w2[e].astype(NP_BF16)
        # re-layout into the exact SBUF tile order so each (slot, hchunk) is
        # ONE contiguous [128, WCOLS] DMA (16KB per partition row)
        w1part = (
            w1f.reshape(NKC, TILE, NHC, HCHUNK)
            .transpose(2, 1, 0, 3)
            .reshape(NHC, TILE, NKC * HCHUNK)
        )
        w2part = (
            w2f.reshape(NHC, NMH, TILE, C)
            .transpose(0, 2, 1, 3)
            .reshape(NHC, TILE, NMH * C)
        )
        wrearr_e[e] = np.ascontiguousarray(
            np.concatenate([w1part, w2part], axis=2)
        )
        wcf = (wc[e] * gamma).astype(NP_BF16)
        wcb_e[e] = np.ascontiguousarray(
            np.broadcast_to(wcf[None, :], (TILE, C))
        )
        bc_e[e] = np.full((TILE, 1), float(bc[e] + float(beta @ wc[e])),
                          dtype=np.float32)
        if not zero_bias:
            b1 = beta @ w1[e]
            b1b_e[e] = np.ascontiguousarray(
                b1.reshape(H // TILE, TILE).T
            ).astype(np.float32)

    in_maps = []
    for c in range(NCORES):
        idx = per_core_idx[c]
        xcrows = np.zeros((M, C), dtype=np.float32)
        valid = idx >= 0
        xcrows[valid] = xf[idx[valid]]
        sl = slot_expert[c]
        m = {
            "xc": xcrows,
            "wr": np.stack([wrearr_e[e] for e in sl]),
            "wcb": np.stack([wcb_e[e] for e in sl]),
            "bcs": np.stack([bc_e[e] for e in sl]),
        }
        if not zero_bias:
            m["b1b"] = np.stack([b1b_e[e] for e in sl])
        in_maps.append(m)

    meta = dict(
        B=B, T=T, C=C, H=H, N=N, M=M, S=S, passes=passes,
        zero_bias=zero_bias, per_core_idx=per_core_idx,
    )
    return in_maps, meta


def _assemble(results, meta):
    N, C = meta["N"], meta["C"]
    out = np.empty((N, C), dtype=np.float32)
    seen = np.zeros(N, dtype=bool)
    for c in range(NCORES):
        idx = meta["per_core_idx"][c]
        valid = idx >= 0
        out[idx[valid]] = results[c]["yc"][valid]
        seen[idx[valid]] = True
    assert seen.all()
    return out.reshape(meta["B"], meta["T"], C)


def kernel_with_results(x, winners, gamma, beta, w1, w2, wc, bc, **run_kwargs):
    in_maps, meta = _prepare(x, winners, gamma, beta, w1, w2, wc, bc)
    nc = _build_program(
        meta["C"], meta["H"], meta["M"], meta["S"], meta["passes"],
        meta["zero_bias"],
    )
    res = run_bass_kernel_spmd(nc, in_maps, core_ids=list(range(NCORES)), **run_kwargs)
    return _assemble(res.results, meta), res


def kernel(x, winners, gamma, beta, w1, w2, wc, bc):
    out, _ = kernel_with_results(x, winners, gamma, beta, w1, w2, wc, bc)
    return out


# revision 52
# speedup vs baseline: 1.0092x; 1.0092x over previous
"""CaMoE block (LayerNorm -> per-expert squared-ReLU FFN with top-1 routing,
confidence-scaled combine, residual) on 8 Trainium2 NeuronCores.

Strategy (token-parallel with expert-grouped tiles):
  * Host: stable-sort tokens by winning expert, pad each expert group to a
    multiple of 128*8 so every core receives the SAME number of 128-token
    tiles per expert. This makes the SPMD program identical across cores
    while every 128-token tile has a single expert.
  * Device (per core): for each 128-token tile: LayerNorm (token-major),
    confidence sigmoid(h.wc+bc) and straight-through scale c/(c+1e-6);
    transpose h via the PE; then stream the expert's W1/W2 in H-chunks and
    run  kT = relu(W1^T h^T)^2  (PE + DVE) and  y += kT^T W2chunk  (PE),
    finally  out = y*scale + x  (DVE) and DMA out.
  * Host: scatter rows back to their original token positions.

gamma/beta of the LayerNorm are folded into W1/wc on the host (plus an
additive H-bias when beta != 0), so the device computes the pre-affine LN.
All matmuls run in bf16 with fp32 PSUM accumulation.
"""

import math
import os
from contextlib import ExitStack

import numpy as np

import concourse.bass as bass
import concourse.mybir as mybir
import concourse.tile as tile
from concourse.bass_utils import run_bass_kernel_spmd
from concourse.masks import make_identity
from concourse.tile import TileContext, ScopedClock

AF = mybir.ActivationFunctionType
OP = mybir.AluOpType
BF16 = mybir.dt.bfloat16
F32 = mybir.dt.float32
NP_BF16 = mybir.dt.np(BF16)

NCORES = 8
TILE = 128
HCHUNK = 512
LN_EPS = 1e-5

# ---------------------------------------------------------------------------
# Workarounds for the walrus build in this environment: it encodes at most
# ONE semaphore wait per instruction and cannot split multi-wait
# instructions itself ("Too many sync wait commands"). We (a) emit the
# TileContext tail-drain waits one-per-NoOp and (b) post-process the whole
# program to hoist excess waits onto same-engine NoOps.
# ---------------------------------------------------------------------------


def _patched_drain_and_barrier(self, tick_clock, wait_clock):
    probe = self.nc.sync.nop(nofuse=True)
    wait_clock.add_sem_waits(probe.ins, ScopedClock({None: tick_clock.global_clock}))
    si = probe.ins.sync_info
    waits = list(si.on_wait) if si is not None and si.on_wait else []
    if len(waits) > 1:
        probe.ins.sync_info = mybir.SyncInfo(on_wait=[waits[0]], on_update=[])
        for w in waits[1:]:
            n = self.nc.sync.nop(nofuse=True)
            n.ins.sync_info = mybir.SyncInfo(on_wait=[w], on_update=[])
    self.nc.sync.drain()
    self.nc.all_engine_barrier()
    assert self.sems is not None
    popped = self.nc._tile_sem_poison_stack.pop()
    assert popped is self._sem_poison
    self.nc.clear_and_free_semaphores(list(self.sems.allocated().values()))
    self.nc.all_engine_barrier()


TileContext._drain_and_barrier = _patched_drain_and_barrier


def _split_excess_waits(nc, max_waits: int = 1):
    for fn in nc.m.functions:
        for bb in fn.blocks:
            insts = list(bb.instructions)
            out = []
            changed = False
            for inst in insts:
                si = inst.sync_info
                waits = list(si.on_wait) if si is not None and si.on_wait else []
                if len(waits) > max_waits:
                    extra = waits[:-max_waits]
                    keep = waits[-max_waits:]
                    for j, w in enumerate(extra):
                        nop = mybir.InstNoOp(
                            name=f"{inst.name}-wsplit{j}", ins=[], outs=[]
                        )
                        nop.engine = inst.engine
                        nop.sync_info = mybir.SyncInfo(on_wait=[w], on_update=[])
                        out.append(nop)
                    inst.sync_info = mybir.SyncInfo(
                        on_wait=keep,
                        on_update=list(si.on_update) if si.on_update else [],
                    )
                    changed = True
                out.append(inst)
            if changed:
                bb.instructions = out


# ---------------------------------------------------------------------------
# Device program
# ---------------------------------------------------------------------------


def _build_program(C, H, M, S, passes, zero_bias):
    """Emit the SPMD Bass program. `passes` is a list of
    (slot, tile_offset, n_tiles<=2); every core runs the same program on its
    own data."""
    NKC = C // TILE          # K-tiles over C (8)
    NMH = HCHUNK // TILE     # M-tiles per H-chunk (4)
    NHC = H // HCHUNK        # H-chunks (8)
    NC2 = C // 512           # output column chunks (2)
    HN = H // TILE           # bias columns (32)

    WCOLS = NKC * HCHUNK + NMH * C  # w1-part then w2-part, tile-contiguous

    nc = bass.Bass("TRN2", target_bir_lowering=False, debug=False)
    xc = nc.dram_tensor("xc", [M, C], F32, kind="ExternalInput").ap()
    wr = nc.dram_tensor("wr", [S, NHC, TILE, WCOLS], BF16, kind="ExternalInput").ap()
    wcb = nc.dram_tensor("wcb", [S, TILE, C], BF16, kind="ExternalInput").ap()
    bcs = nc.dram_tensor("bcs", [S, TILE, 1], F32, kind="ExternalInput").ap()
    if not zero_bias:
        b1b = nc.dram_tensor("b1b", [S, TILE, HN], F32, kind="ExternalInput").ap()
    yc = nc.dram_tensor("yc", [M, C], F32, kind="ExternalOutput").ap()

    with TileContext(nc) as tc, ExitStack() as ctx:
        cpool = ctx.enter_context(tc.tile_pool(name="const", bufs=1))
        ident = cpool.tile([TILE, TILE], BF16, tag="ident")
        make_identity(nc, ident[:])
        epsc = cpool.tile([TILE, 1], F32, tag="eps")
        nc.gpsimd.memset(epsc[:], LN_EPS)

        # weights stay RESIDENT for a whole slot (8 chunks x 2MB); the pool
        # rotation naturally overlaps the next slot's loads with the current
        # slot's last-pass reads
        wpool = ctx.enter_context(tc.tile_pool(name="w", bufs=8))
        spool = ctx.enter_context(tc.tile_pool(name="slot", bufs=2))
        xpool = ctx.enter_context(tc.tile_pool(name="x", bufs=4))
        hpool = ctx.enter_context(tc.tile_pool(name="h", bufs=4))
        prpool = ctx.enter_context(tc.tile_pool(name="pr", bufs=2))
        htpool = ctx.enter_context(tc.tile_pool(name="ht", bufs=2))
        kpool = ctx.enter_context(tc.tile_pool(name="kt", bufs=4))
        opool = ctx.enter_context(tc.tile_pool(name="o", bufs=3))
        stpool = ctx.enter_context(tc.tile_pool(name="st", bufs=8))
        sqpool = ctx.enter_context(tc.tile_pool(name="sq", bufs=1))
        pps = ctx.enter_context(tc.tile_pool(name="pk", bufs=2, space="PSUM"))
        ppy = ctx.enter_context(tc.tile_pool(name="py", bufs=4, space="PSUM"))
        ppt = ctx.enter_context(tc.tile_pool(name="ptr", bufs=2, space="PSUM"))

        slot_consts = {}
        w_chunks = {}

        def get_slot_consts(si):
            # NOTE: spool bufs must cover the number of distinct slots alive
            # at once (current + next pass's). Entries are invalidated by the
            # pool's slot reuse; with bufs=2 and passes grouped by slot this
            # holds.
            if si not in slot_consts:
                wcb_sb = spool.tile([TILE, C], BF16, tag="wcb", name=f"wcb{si}")
                nc.sync.dma_start(wcb_sb[:], wcb[si])
                bct = spool.tile([TILE, 1], F32, tag="bc", name=f"bc{si}")
                nc.sync.dma_start(bct[:], bcs[si])
                b1_sb = None
                if not zero_bias:
                    b1_sb = spool.tile([TILE, HN], F32, tag="b1", name=f"b1{si}")
                    nc.sync.dma_start(b1_sb[:], b1b[si])
                slot_consts[si] = (wcb_sb, bct, b1_sb)
            return slot_consts[si]

        def emit_stage_a(pass_idx):
            si, tile_off, nt = passes[pass_idx]
            ntok = TILE * nt
            wcb_sb, bct, _ = get_slot_consts(si)
            x_t = []
            s_t = []
            hT = htpool.tile([TILE, NKC, ntok], BF16, tag="hT",
                             name=f"hT{pass_idx}")
            # ops are interleaved across the pass's tiles so the serial
            # dep-chain latency (with a sem hop per tiny op) is paid once,
            # not once per tile
            st = lambda tag: [stpool.tile([TILE, 1], F32, tag=tag, name=f"{tag}{pass_idx}_{t}") for t in range(nt)]
            for t in range(nt):
                row0 = (tile_off + t) * TILE
                xt = xpool.tile([TILE, C], F32, tag="x", name=f"x{pass_idx}_{t}")
                x_t.append(xt)
                nc.sync.dma_start(xt[:], xc[row0 : row0 + TILE, :])
            nsum, negmu, ssq, std, rs, nmrs = (
                st("nsum"), st("negmu"), st("ssq"), st("std"), st("rs"),
                st("nmrs"),
            )
            cdot, conf, cpe, rc = st("cdot"), st("conf"), st("cpe"), st("rc")
            hts = []
            for t in range(nt):
                nc.vector.reduce_sum(
                    nsum[t][:], x_t[t][:], axis=mybir.AxisListType.X,
                    negate=True,
                )
            for t in range(nt):
                nc.scalar.mul(negmu[t][:], nsum[t][:], 1.0 / C)
            if zero_bias:
                # Fast path: transpose xc = x - mu (available right after the
                # mean); 1/std commutes through relu^2 and folds into the
                # stage-C per-token scalar:
                #   relu(rs*(xc@W1))^2 @ W2 = rs^2 * (relu(xc@W1)^2 @ W2)
                for t in range(nt):
                    ht_ = hpool.tile([TILE, C], BF16, tag="h",
                                     name=f"h{pass_idx}_{t}")
                    hts.append(ht_)
                    nc.scalar.activation(
                        ht_[:], x_t[t][:], AF.Identity, bias=negmu[t][:],
                        scale=1.0,
                    )
                for t in range(nt):
                    sq = sqpool.tile([TILE, C], F32, tag="sq")
                    nc.scalar.activation(
                        sq[:], x_t[t][:], AF.Square, bias=negmu[t][:],
                        scale=1.0, accum_out=ssq[t][:],
                    )
                for t in range(nt):
                    nc.scalar.activation(
                        std[t][:], ssq[t][:], AF.Sqrt, bias=epsc[:],
                        scale=1.0 / C,
                    )
                for t in range(nt):
                    nc.vector.reciprocal(rs[t][:], std[t][:])
                for t in range(nt):
                    prod = prpool.tile([TILE, C], BF16, tag="prod")
                    nc.vector.scalar_tensor_tensor(
                        prod[:], hts[t][:], 1.0, wcb_sb[:], op0=OP.mult,
                        op1=OP.mult, accum_out=cdot[t][:],
                    )
                for t in range(nt):
                    nc.scalar.activation(
                        conf[t][:], cdot[t][:], AF.Sigmoid, bias=bct[:],
                        scale=rs[t][:],
                    )
                for t in range(nt):
                    nc.vector.tensor_scalar_add(cpe[t][:], conf[t][:], 1e-6)
                for t in range(nt):
                    nc.vector.reciprocal(rc[t][:], cpe[t][:])
                rs2 = st("rs2")
                sc0 = st("sc0")
                for t in range(nt):
                    nc.vector.tensor_mul(rs2[t][:], rs[t][:], rs[t][:])
                for t in range(nt):
                    nc.vector.tensor_mul(sc0[t][:], conf[t][:], rc[t][:])
                for t in range(nt):
                    sc = stpool.tile([TILE, 1], F32, tag="sc",
                                     name=f"sc{pass_idx}_{t}")
                    nc.vector.tensor_mul(sc[:], sc0[t][:], rs2[t][:])
                    s_t.append(sc)
                return x_t, s_t, hT, hts
            for t in range(nt):
                sq = sqpool.tile([TILE, C], F32, tag="sq")
                nc.scalar.activation(
                    sq[:], x_t[t][:], AF.Square, bias=negmu[t][:], scale=1.0,
                    accum_out=ssq[t][:],
                )
            for t in range(nt):
                nc.scalar.activation(
                    std[t][:], ssq[t][:], AF.Sqrt, bias=epsc[:], scale=1.0 / C
                )
            for t in range(nt):
                nc.vector.reciprocal(rs[t][:], std[t][:])
            for t in range(nt):
                nc.vector.tensor_mul(nmrs[t][:], negmu[t][:], rs[t][:])
            for t in range(nt):
                ht_ = hpool.tile([TILE, C], BF16, tag="h", name=f"h{pass_idx}_{t}")
                hts.append(ht_)
                nc.scalar.activation(
                    ht_[:], x_t[t][:], AF.Identity, bias=nmrs[t][:],
                    scale=rs[t][:],
                )
            for t in range(nt):
                prod = prpool.tile([TILE, C], BF16, tag="prod")
                nc.vector.scalar_tensor_tensor(
                    prod[:], hts[t][:], 1.0, wcb_sb[:], op0=OP.mult,
                    op1=OP.mult, accum_out=cdot[t][:],
                )
            for t in range(nt):
                nc.scalar.activation(
                    conf[t][:], cdot[t][:], AF.Sigmoid, bias=bct[:], scale=1.0
                )
            for t in range(nt):
                nc.vector.tensor_scalar_add(cpe[t][:], conf[t][:], 1e-6)
            for t in range(nt):
                nc.vector.reciprocal(rc[t][:], cpe[t][:])
            for t in range(nt):
                sc = stpool.tile([TILE, 1], F32, tag="sc", name=f"sc{pass_idx}_{t}")
                nc.vector.tensor_mul(sc[:], conf[t][:], rc[t][:])
                s_t.append(sc)
            return x_t, s_t, hT, hts

        def emit_transposes(pass_idx):
            # phase 2: emitted well after phase 1 so the DVE copies never
            # head-of-line block the relu^2 stream waiting on the LN chain
            _, _, nt = passes[pass_idx]
            x_t, s_t, hT, hts = stage_a[pass_idx]
            for t in range(nt):
                for kc in range(NKC):
                    pt = ppt.tile([TILE, TILE], BF16, tag="ptr")
                    nc.tensor.transpose(
                        pt[:], hts[t][:, kc * TILE : (kc + 1) * TILE], ident[:]
                    )
                    nc.vector.tensor_copy(
                        hT[:, kc, t * TILE : (t + 1) * TILE], pt[:]
                    )

        stage_a = {}
        transposed = set()

        def get_stage_a(pass_idx):
            if pass_idx < len(passes) and pass_idx not in stage_a:
                stage_a[pass_idx] = emit_stage_a(pass_idx)
            return stage_a.get(pass_idx)

        def get_transposes(pass_idx):
            if pass_idx < len(passes) and pass_idx not in transposed:
                get_stage_a(pass_idx)
                transposed.add(pass_idx)
                emit_transposes(pass_idx)

        pending_stage_c = []
        for pass_idx, (si, tile_off, nt) in enumerate(passes):
            ntok = TILE * nt
            get_transposes(pass_idx)
            x_t, s_t, hT, _ = stage_a[pass_idx]
            _, _, b1_sb = get_slot_consts(si)

            ys = [
                ppy.tile([TILE, 512], F32, tag="py", name=f"ys{i}")
                for i in range(nt * NC2)
            ]

            def emit_mm2(hc, mh, kt, wt):
                w2base = NKC * HCHUNK + mh * C
                for t in range(nt):
                    for ncx in range(NC2):
                        nc.tensor.matmul(
                            ys[t * NC2 + ncx][:],
                            kt[:, t * TILE : (t + 1) * TILE],
                            wt[:, w2base + ncx * 512 : w2base + (ncx + 1) * 512],
                            start=(hc == 0 and mh == 0),
                            stop=(hc == NHC - 1 and mh == NMH - 1),
                        )

            # mm2(g) depends on the ACT/DVE relu^2 of mm1(g)'s psum, and the
            # first mm2 of a pass also waits on the previous pass's combine
            # (ys release); the PE is in-order, so emit mm2(g) two mm1
            # groups later to hide both latencies.
            pending = []  # [(hc, mh, kt, wt), ...]
            for hc in range(NHC):
                if (si, hc) not in w_chunks:
                    wt = wpool.tile([TILE, WCOLS], BF16, tag="w",
                                    name=f"w{si}_{hc}")
                    nc.sync.dma_start(wt[:], wr[si, hc])
                    w_chunks[(si, hc)] = wt
                wt = w_chunks[(si, hc)]
                if hc == 1:
                    # next pass's LayerNorm/conf: in-order ACT/DVE streams
                    # reach them mid-pass instead of after all relu ops
                    get_stage_a(pass_idx + 1)
                if hc == 4:
                    get_transposes(pass_idx + 1)
                for mh in range(NMH):
                    pk = pps.tile([TILE, ntok], F32, tag="pk")
                    for kc in range(NKC):
                        nc.tensor.matmul(
                            pk[:],
                            wt[:, kc * HCHUNK + mh * TILE : kc * HCHUNK + (mh + 1) * TILE],
                            hT[:, kc, :],
                            start=(kc == 0),
                            stop=(kc == NKC - 1),
                        )
                    kt = kpool.tile([TILE, ntok], BF16, tag="kt")
                    kr = kpool.tile([TILE, ntok], BF16, tag="kr")
                    if zero_bias:
                        # relu on DVE keeps the mm2-feeding path off the
                        # (LayerNorm-busy) ACT engine
                        nc.vector.tensor_scalar_max(kr[:], pk[:], 0.0)
                    else:
                        col = hc * NMH + mh
                        nc.scalar.activation(
                            kr[:], pk[:], AF.Relu,
                            bias=b1_sb[:, col : col + 1], scale=1.0,
                        )
                    nc.vector.tensor_mul(kt[:], kr[:], kr[:])
                    pending.append((hc, mh, kt, wt))
                    if hc == 0 and pending_stage_c:
                        # previous pass's combine, one tile per mm1 group:
                        # frees that pass's ys psum banks incrementally right
                        # behind this pass's first relus
                        pending_stage_c.pop(0)()
                    if len(pending) > 2:
                        emit_mm2(*pending.pop(0))
            while pending:
                emit_mm2(*pending.pop(0))

            def stage_c_tile(t, ys=ys, x_t=x_t, s_t=s_t, tile_off=tile_off):
                row0 = (tile_off + t) * TILE
                ot = opool.tile([TILE, C], F32, tag="o")
                for ncx in range(NC2):
                    nc.vector.scalar_tensor_tensor(
                        ot[:, ncx * 512 : (ncx + 1) * 512],
                        ys[t * NC2 + ncx][:],
                        s_t[t][:],
                        x_t[t][:, ncx * 512 : (ncx + 1) * 512],
                        op0=OP.mult,
                        op1=OP.add,
                    )
                nc.sync.dma_start(yc[row0 : row0 + TILE, :], ot[:])

            while pending_stage_c:
                pending_stage_c.pop(0)()
            pending_stage_c.extend(
                (lambda t=t: stage_c_tile(t)) for t in range(nt)
            )
        while pending_stage_c:
            pending_stage_c.pop(0)()

    _split_excess_waits(nc, 1)
    return nc


# ---------------------------------------------------------------------------
# Host-side dispatch
# ---------------------------------------------------------------------------


def _partitions(total, max_part, max_len):
    if total == 0:
        yield ()
        return
    if max_len == 0:
        return
    for first in range(min(total, max_part), 0, -1):
        for rest in _partitions(total - first, first, max_len - 1):
            yield (first,) + rest


def _try_pack(tiles, Tvec):
    """Greedy: assign each expert (desc) slot instances (8 per slot type).
    Returns assign list aligned with `tiles` order, or None."""
    avail = [list(range(NCORES)) for _ in Tvec]
    order_i = sorted(range(len(tiles)), key=lambda i: -tiles[i])
    assign = [None] * len(tiles)
    sizes = sorted(range(len(Tvec)), key=lambda j: -Tvec[j])
    for i in order_i:
        rem = tiles[i]
        inst = []
        while rem > 0:
            # largest slot type with size <= rem, else smallest type >= rem
            pick = None
            for j in sizes:
                if avail[j] and Tvec[j] <= rem:
                    pick = j
                    break
            if pick is None:
                for j in reversed(sizes):
                    if avail[j]:
                        pick = j
                        break
            if pick is None:
                return None
            c = avail[pick].pop(0)
            inst.append((pick, c))
            rem -= Tvec[pick]
        assign[i] = inst
    return assign


def _pack_slots(tiles):
    """Choose per-core slot sizes Tvec (identical structure on all cores)
    and an (expert -> slot instances) assignment minimizing per-core tiles."""
    total = sum(tiles)
    pmin = int(math.ceil(total / NCORES))
    for P in range(pmin, pmin + 4):
        cands = sorted(_partitions(P, P, 6), key=len)
        for Tvec in cands:
            a = _try_pack(tiles, list(Tvec))
            if a is not None:
                return list(Tvec), a
    # fallback: one dedicated slot per expert on every core
    Tvec = [int(math.ceil(t / NCORES)) for t in tiles]
    assign = [[(j, c) for c in range(NCORES)] for j in range(len(tiles))]
    return Tvec, assign


def _prepare(x, winners, gamma, beta, w1, w2, wc, bc):
    x = np.ascontiguousarray(np.asarray(x, dtype=np.float32))
    winners = np.asarray(winners).reshape(-1).astype(np.int64)
    gamma = np.asarray(gamma, dtype=np.float32)
    beta = np.asarray(beta, dtype=np.float32)
    w1 = np.asarray(w1, dtype=np.float32)
    w2 = np.asarray(w2, dtype=np.float32)
    wc = np.asarray(wc, dtype=np.float32)
    bc = np.asarray(bc, dtype=np.float32)

    B, T, C = x.shape
    E, _, H = w1.shape
    N = B * T
    xf = x.reshape(N, C)

    order = np.argsort(winners, kind="stable")
    counts = np.bincount(winners, minlength=E)

    present = [e for e in range(E) if counts[e] > 0]
    tiles_e = {e: int(math.ceil(counts[e] / TILE)) for e in present}

    Tvec, assign = _pack_slots([tiles_e[e] for e in present])
    # assign[i] = list of (slot_j, core_c) instances for present[i]
    S = len(Tvec)

    # slot_expert[c][j]: which expert's weights core c streams in slot j
    # (dummy instances reuse expert present[0]'s weights; their tokens are 0)
    slot_expert = [[present[0]] * S for _ in range(NCORES)]
    # token index list per (core, slot): length Tvec[j]*TILE, -1 = padding
    slot_idx = [
        [np.full(Tvec[j] * TILE, -1, dtype=np.int64) for j in range(S)]
        for c in range(NCORES)
    ]
    pos = 0
    for i, e in enumerate(present):
        n_e = int(counts[e])
        toks = order[pos : pos + n_e]
        pos += n_e
        filled = 0
        for (j, c) in assign[i]:
            slot_expert[c][j] = e
            cap = Tvec[j] * TILE
            take = min(cap, n_e - filled)
            if take > 0:
                slot_idx[c][j][:take] = toks[filled : filled + take]
                filled += take
        assert filled == n_e

    per_core_idx = [np.concatenate(slot_idx[c]) for c in range(NCORES)]
    M = per_core_idx[0].size

    passes = []
    tile_off = 0
    for j in range(S):
        k = 0
        while k < Tvec[j]:
            nt = min(2, Tvec[j] - k)
            passes.append((j, tile_off + k, nt))
            k += nt
        tile_off += Tvec[j]

    # fold gamma/beta; build per-EXPERT folded tensors once, then per-core
    # stacks indexed by that core's slot->expert table
    NKC = C // TILE
    NMH = HCHUNK // TILE
    NHC = H // HCHUNK
    zero_bias = bool(np.all(beta == 0.0))
    wrearr_e = {}
    wcb_e = {}
    bc_e = {}
    b1b_e = {}
    for e in present:
        w1f = (w1[e] * gamma[:, None]).astype(NP_BF16)
        w2f = w2[e].astype(NP_BF16)
        # re-layout into the exact SBUF tile order so each (slot, hchunk) is
        # ONE contiguous [128, WCOLS] DMA (16KB per partition row)
        w1part = (
            w1f.reshape(NKC, TILE, NHC, HCHUNK)
            .transpose(2, 1, 0, 3)
            .reshape(NHC, TILE, NKC * HCHUNK)
        )
        w2part = (
            w2f.reshape(NHC, NMH, TILE, C)
            .transpose(0, 2, 1, 3)
            .reshape(NHC, TILE, NMH * C)
        )
        wrearr_e[e] = np.ascontiguousarray(
            np.concatenate([w1part, w2part], axis=2)
        )
        wcf = (wc[e] * gamma).astype(NP_BF16)
        wcb_e[e] = np.ascontiguousarray(
            np.broadcast_to(wcf[None, :], (TILE, C))
        )
        bc_e[e] = np.full((TILE, 1), float(bc[e] + float(beta @ wc[e])),
                          dtype=np.float32)
        if not zero_bias:
            b1 = beta @ w1[e]
            b1b_e[e] = np.ascontiguousarray(
                b1.reshape(H // TILE, TILE).T
            ).astype(np.float32)

    in_maps = []
    for c in range(NCORES):
        idx = per_core_idx[c]
        xcrows = np.zeros((M, C), dtype=np.float32)
        valid = idx >= 0
        xcrows[valid] = xf[idx[valid]]
        sl = slot_expert[c]
        m = {
            "xc": xcrows,
            "wr": np.stack([wrearr_e[e] for e in sl]),
            "wcb": np.stack([wcb_e[e] for e in sl]),
            "bcs": np.stack([bc_e[e] for e in sl]),
        }
        if not zero_bias:
            m["b1b"] = np.stack([b1b_e[e] for e in sl])
        in_maps.append(m)

    meta = dict(
        B=B, T=T, C=C, H=H, N=N, M=M, S=S, passes=passes,
        zero_bias=zero_bias, per_core_idx=per_core_idx,
    )
    return in_maps, meta


def _assemble(results, meta):
    N, C = meta["N"], meta["C"]
    out = np.empty((N, C), dtype=np.float32)
    seen = np.zeros(N, dtype=bool)
    for c in range(NCORES):
        idx = meta["per_core_idx"][c]
        valid = idx >= 0
        out[idx[valid]] = results[c]["yc"][valid]
        seen[idx[valid]] = True
    assert seen.all()
    return out.reshape(meta["B"], meta["T"], C)


def kernel_with_results(x, winners, gamma, beta, w1, w2, wc, bc, **run_kwargs):
    in_maps, meta = _prepare(x, winners, gamma, beta, w1, w2, wc, bc)
    nc = _build_program(
        meta["C"], meta["H"], meta["M"], meta["S"], meta["passes"],
        meta["zero_bias"],
    )
    res = run_bass_kernel_spmd(nc, in_maps, core_ids=list(range(NCORES)), **run_kwargs)
    return _assemble(res.results, meta), res


def kernel(x, winners, gamma, beta, w1, w2, wc, bc):
    out, _ = kernel_with_results(x, winners, gamma, beta, w1, w2, wc, bc)
    return out


# revision 53
# speedup vs baseline: 1.0181x; 1.0088x over previous
"""CaMoE block (LayerNorm -> per-expert squared-ReLU FFN with top-1 routing,
confidence-scaled combine, residual) on 8 Trainium2 NeuronCores.

Strategy (token-parallel with expert-grouped tiles):
  * Host: stable-sort tokens by winning expert, pad each expert group to a
    multiple of 128*8 so every core receives the SAME number of 128-token
    tiles per expert. This makes the SPMD program identical across cores
    while every 128-token tile has a single expert.
  * Device (per core): for each 128-token tile: LayerNorm (token-major),
    confidence sigmoid(h.wc+bc) and straight-through scale c/(c+1e-6);
    transpose h via the PE; then stream the expert's W1/W2 in H-chunks and
    run  kT = relu(W1^T h^T)^2  (PE + DVE) and  y += kT^T W2chunk  (PE),
    finally  out = y*scale + x  (DVE) and DMA out.
  * Host: scatter rows back to their original token positions.

gamma/beta of the LayerNorm are folded into W1/wc on the host (plus an
additive H-bias when beta != 0), so the device computes the pre-affine LN.
All matmuls run in bf16 with fp32 PSUM accumulation.
"""

import math
import os
from contextlib import ExitStack

import numpy as np

import concourse.bass as bass
import concourse.mybir as mybir
import concourse.tile as tile
from concourse.bass_utils import run_bass_kernel_spmd
from concourse.masks import make_identity
from concourse.tile import TileContext, ScopedClock

AF = mybir.ActivationFunctionType
OP = mybir.AluOpType
BF16 = mybir.dt.bfloat16
F32 = mybir.dt.float32
NP_BF16 = mybir.dt.np(BF16)

NCORES = 8
TILE = 128
HCHUNK = 1024
LN_EPS = 1e-5

# ---------------------------------------------------------------------------
# Workarounds for the walrus build in this environment: it encodes at most
# ONE semaphore wait per instruction and cannot split multi-wait
# instructions itself ("Too many sync wait commands"). We (a) emit the
# TileContext tail-drain waits one-per-NoOp and (b) post-process the whole
# program to hoist excess waits onto same-engine NoOps.
# ---------------------------------------------------------------------------


def _patched_drain_and_barrier(self, tick_clock, wait_clock):
    probe = self.nc.sync.nop(nofuse=True)
    wait_clock.add_sem_waits(probe.ins, ScopedClock({None: tick_clock.global_clock}))
    si = probe.ins.sync_info
    waits = list(si.on_wait) if si is not None and si.on_wait else []
    if len(waits) > 1:
        probe.ins.sync_info = mybir.SyncInfo(on_wait=[waits[0]], on_update=[])
        for w in waits[1:]:
            n = self.nc.sync.nop(nofuse=True)
            n.ins.sync_info = mybir.SyncInfo(on_wait=[w], on_update=[])
    self.nc.sync.drain()
    self.nc.all_engine_barrier()
    assert self.sems is not None
    popped = self.nc._tile_sem_poison_stack.pop()
    assert popped is self._sem_poison
    self.nc.clear_and_free_semaphores(list(self.sems.allocated().values()))
    self.nc.all_engine_barrier()


TileContext._drain_and_barrier = _patched_drain_and_barrier


def _split_excess_waits(nc, max_waits: int = 1):
    for fn in nc.m.functions:
        for bb in fn.blocks:
            insts = list(bb.instructions)
            out = []
            changed = False
            for inst in insts:
                si = inst.sync_info
                waits = list(si.on_wait) if si is not None and si.on_wait else []
                if len(waits) > max_waits:
                    extra = waits[:-max_waits]
                    keep = waits[-max_waits:]
                    for j, w in enumerate(extra):
                        nop = mybir.InstNoOp(
                            name=f"{inst.name}-wsplit{j}", ins=[], outs=[]
                        )
                        nop.engine = inst.engine
                        nop.sync_info = mybir.SyncInfo(on_wait=[w], on_update=[])
                        out.append(nop)
                    inst.sync_info = mybir.SyncInfo(
                        on_wait=keep,
                        on_update=list(si.on_update) if si.on_update else [],
                    )
                    changed = True
                out.append(inst)
            if changed:
                bb.instructions = out


# ---------------------------------------------------------------------------
# Device program
# ---------------------------------------------------------------------------


def _build_program(C, H, M, S, passes, zero_bias):
    """Emit the SPMD Bass program. `passes` is a list of
    (slot, tile_offset, n_tiles<=2); every core runs the same program on its
    own data."""
    NKC = C // TILE          # K-tiles over C (8)
    NMH = HCHUNK // TILE     # M-tiles per H-chunk (4)
    NHC = H // HCHUNK        # H-chunks (8)
    NC2 = C // 512           # output column chunks (2)
    HN = H // TILE           # bias columns (32)

    WCOLS = NKC * HCHUNK + NMH * C  # w1-part then w2-part, tile-contiguous

    nc = bass.Bass("TRN2", target_bir_lowering=False, debug=False)
    xc = nc.dram_tensor("xc", [M, C], F32, kind="ExternalInput").ap()
    wr = nc.dram_tensor("wr", [S, NHC, TILE, WCOLS], BF16, kind="ExternalInput").ap()
    wcb = nc.dram_tensor("wcb", [S, TILE, C], BF16, kind="ExternalInput").ap()
    bcs = nc.dram_tensor("bcs", [S, TILE, 1], F32, kind="ExternalInput").ap()
    if not zero_bias:
        b1b = nc.dram_tensor("b1b", [S, TILE, HN], F32, kind="ExternalInput").ap()
    yc = nc.dram_tensor("yc", [M, C], F32, kind="ExternalOutput").ap()

    with TileContext(nc) as tc, ExitStack() as ctx:
        cpool = ctx.enter_context(tc.tile_pool(name="const", bufs=1))
        ident = cpool.tile([TILE, TILE], BF16, tag="ident")
        make_identity(nc, ident[:])
        epsc = cpool.tile([TILE, 1], F32, tag="eps")
        nc.gpsimd.memset(epsc[:], LN_EPS)

        # weights stay RESIDENT for a whole slot (8 chunks x 2MB); the pool
        # rotation naturally overlaps the next slot's loads with the current
        # slot's last-pass reads
        wpool = ctx.enter_context(tc.tile_pool(name="w", bufs=4))
        spool = ctx.enter_context(tc.tile_pool(name="slot", bufs=2))
        xpool = ctx.enter_context(tc.tile_pool(name="x", bufs=4))
        hpool = ctx.enter_context(tc.tile_pool(name="h", bufs=4))
        prpool = ctx.enter_context(tc.tile_pool(name="pr", bufs=2))
        htpool = ctx.enter_context(tc.tile_pool(name="ht", bufs=2))
        kpool = ctx.enter_context(tc.tile_pool(name="kt", bufs=4))
        opool = ctx.enter_context(tc.tile_pool(name="o", bufs=3))
        stpool = ctx.enter_context(tc.tile_pool(name="st", bufs=8))
        sqpool = ctx.enter_context(tc.tile_pool(name="sq", bufs=1))
        pps = ctx.enter_context(tc.tile_pool(name="pk", bufs=2, space="PSUM"))
        ppy = ctx.enter_context(tc.tile_pool(name="py", bufs=4, space="PSUM"))
        ppt = ctx.enter_context(tc.tile_pool(name="ptr", bufs=2, space="PSUM"))

        slot_consts = {}
        w_chunks = {}

        def get_slot_consts(si):
            # NOTE: spool bufs must cover the number of distinct slots alive
            # at once (current + next pass's). Entries are invalidated by the
            # pool's slot reuse; with bufs=2 and passes grouped by slot this
            # holds.
            if si not in slot_consts:
                wcb_sb = spool.tile([TILE, C], BF16, tag="wcb", name=f"wcb{si}")
                nc.sync.dma_start(wcb_sb[:], wcb[si])
                bct = spool.tile([TILE, 1], F32, tag="bc", name=f"bc{si}")
                nc.sync.dma_start(bct[:], bcs[si])
                b1_sb = None
                if not zero_bias:
                    b1_sb = spool.tile([TILE, HN], F32, tag="b1", name=f"b1{si}")
                    nc.sync.dma_start(b1_sb[:], b1b[si])
                slot_consts[si] = (wcb_sb, bct, b1_sb)
            return slot_consts[si]

        def emit_stage_a(pass_idx):
            si, tile_off, nt = passes[pass_idx]
            ntok = TILE * nt
            wcb_sb, bct, _ = get_slot_consts(si)
            x_t = []
            s_t = []
            hT = htpool.tile([TILE, NKC, ntok], BF16, tag="hT",
                             name=f"hT{pass_idx}")
            # ops are interleaved across the pass's tiles so the serial
            # dep-chain latency (with a sem hop per tiny op) is paid once,
            # not once per tile
            st = lambda tag: [stpool.tile([TILE, 1], F32, tag=tag, name=f"{tag}{pass_idx}_{t}") for t in range(nt)]
            for t in range(nt):
                row0 = (tile_off + t) * TILE
                xt = xpool.tile([TILE, C], F32, tag="x", name=f"x{pass_idx}_{t}")
                x_t.append(xt)
                nc.sync.dma_start(xt[:], xc[row0 : row0 + TILE, :])
            nsum, negmu, ssq, std, rs, nmrs = (
                st("nsum"), st("negmu"), st("ssq"), st("std"), st("rs"),
                st("nmrs"),
            )
            cdot, conf, cpe, rc = st("cdot"), st("conf"), st("cpe"), st("rc")
            hts = []
            for t in range(nt):
                nc.vector.reduce_sum(
                    nsum[t][:], x_t[t][:], axis=mybir.AxisListType.X,
                    negate=True,
                )
            for t in range(nt):
                nc.scalar.mul(negmu[t][:], nsum[t][:], 1.0 / C)
            if zero_bias:
                # Fast path: transpose xc = x - mu (available right after the
                # mean); 1/std commutes through relu^2 and folds into the
                # stage-C per-token scalar:
                #   relu(rs*(xc@W1))^2 @ W2 = rs^2 * (relu(xc@W1)^2 @ W2)
                for t in range(nt):
                    ht_ = hpool.tile([TILE, C], BF16, tag="h",
                                     name=f"h{pass_idx}_{t}")
                    hts.append(ht_)
                    nc.scalar.activation(
                        ht_[:], x_t[t][:], AF.Identity, bias=negmu[t][:],
                        scale=1.0,
                    )
                for t in range(nt):
                    sq = sqpool.tile([TILE, C], F32, tag="sq")
                    nc.scalar.activation(
                        sq[:], x_t[t][:], AF.Square, bias=negmu[t][:],
                        scale=1.0, accum_out=ssq[t][:],
                    )
                for t in range(nt):
                    nc.scalar.activation(
                        std[t][:], ssq[t][:], AF.Sqrt, bias=epsc[:],
                        scale=1.0 / C,
                    )
                for t in range(nt):
                    nc.vector.reciprocal(rs[t][:], std[t][:])
                for t in range(nt):
                    prod = prpool.tile([TILE, C], BF16, tag="prod")
                    nc.vector.scalar_tensor_tensor(
                        prod[:], hts[t][:], 1.0, wcb_sb[:], op0=OP.mult,
                        op1=OP.mult, accum_out=cdot[t][:],
                    )
                for t in range(nt):
                    nc.scalar.activation(
                        conf[t][:], cdot[t][:], AF.Sigmoid, bias=bct[:],
                        scale=rs[t][:],
                    )
                for t in range(nt):
                    nc.vector.tensor_scalar_add(cpe[t][:], conf[t][:], 1e-6)
                for t in range(nt):
                    nc.vector.reciprocal(rc[t][:], cpe[t][:])
                rs2 = st("rs2")
                sc0 = st("sc0")
                for t in range(nt):
                    nc.vector.tensor_mul(rs2[t][:], rs[t][:], rs[t][:])
                for t in range(nt):
                    nc.vector.tensor_mul(sc0[t][:], conf[t][:], rc[t][:])
                for t in range(nt):
                    sc = stpool.tile([TILE, 1], F32, tag="sc",
                                     name=f"sc{pass_idx}_{t}")
                    nc.vector.tensor_mul(sc[:], sc0[t][:], rs2[t][:])
                    s_t.append(sc)
                return x_t, s_t, hT, hts
            for t in range(nt):
                sq = sqpool.tile([TILE, C], F32, tag="sq")
                nc.scalar.activation(
                    sq[:], x_t[t][:], AF.Square, bias=negmu[t][:], scale=1.0,
                    accum_out=ssq[t][:],
                )
            for t in range(nt):
                nc.scalar.activation(
                    std[t][:], ssq[t][:], AF.Sqrt, bias=epsc[:], scale=1.0 / C
                )
            for t in range(nt):
                nc.vector.reciprocal(rs[t][:], std[t][:])
            for t in range(nt):
                nc.vector.tensor_mul(nmrs[t][:], negmu[t][:], rs[t][:])
            for t in range(nt):
                ht_ = hpool.tile([TILE, C], BF16, tag="h", name=f"h{pass_idx}_{t}")
                hts.append(ht_)
                nc.scalar.activation(
                    ht_[:], x_t[t][:], AF.Identity, bias=nmrs[t][:],
                    scale=rs[t][:],
                )
            for t in range(nt):
                prod = prpool.tile([TILE, C], BF16, tag="prod")
                nc.vector.scalar_tensor_tensor(
                    prod[:], hts[t][:], 1.0, wcb_sb[:], op0=OP.mult,
                    op1=OP.mult, accum_out=cdot[t][:],
                )
            for t in range(nt):
                nc.scalar.activation(
                    conf[t][:], cdot[t][:], AF.Sigmoid, bias=bct[:], scale=1.0
                )
            for t in range(nt):
                nc.vector.tensor_scalar_add(cpe[t][:], conf[t][:], 1e-6)
            for t in range(nt):
                nc.vector.reciprocal(rc[t][:], cpe[t][:])
            for t in range(nt):
                sc = stpool.tile([TILE, 1], F32, tag="sc", name=f"sc{pass_idx}_{t}")
                nc.vector.tensor_mul(sc[:], conf[t][:], rc[t][:])
                s_t.append(sc)
            return x_t, s_t, hT, hts

        def emit_transposes(pass_idx):
            # phase 2: emitted well after phase 1 so the DVE copies never
            # head-of-line block the relu^2 stream waiting on the LN chain
            _, _, nt = passes[pass_idx]
            x_t, s_t, hT, hts = stage_a[pass_idx]
            for t in range(nt):
                for kc in range(NKC):
                    pt = ppt.tile([TILE, TILE], BF16, tag="ptr")
                    nc.tensor.transpose(
                        pt[:], hts[t][:, kc * TILE : (kc + 1) * TILE], ident[:]
                    )
                    nc.vector.tensor_copy(
                        hT[:, kc, t * TILE : (t + 1) * TILE], pt[:]
                    )

        stage_a = {}
        transposed = set()

        def get_stage_a(pass_idx):
            if pass_idx < len(passes) and pass_idx not in stage_a:
                stage_a[pass_idx] = emit_stage_a(pass_idx)
            return stage_a.get(pass_idx)

        def get_transposes(pass_idx):
            if pass_idx < len(passes) and pass_idx not in transposed:
                get_stage_a(pass_idx)
                transposed.add(pass_idx)
                emit_transposes(pass_idx)

        pending_stage_c = []
        for pass_idx, (si, tile_off, nt) in enumerate(passes):
            ntok = TILE * nt
            get_transposes(pass_idx)
            x_t, s_t, hT, _ = stage_a[pass_idx]
            _, _, b1_sb = get_slot_consts(si)

            ys = [
                ppy.tile([TILE, 512], F32, tag="py", name=f"ys{i}")
                for i in range(nt * NC2)
            ]

            def emit_mm2(hc, mh, kt, wt):
                w2base = NKC * HCHUNK + mh * C
                for t in range(nt):
                    for ncx in range(NC2):
                        nc.tensor.matmul(
                            ys[t * NC2 + ncx][:],
                            kt[:, t * TILE : (t + 1) * TILE],
                            wt[:, w2base + ncx * 512 : w2base + (ncx + 1) * 512],
                            start=(hc == 0 and mh == 0),
                            stop=(hc == NHC - 1 and mh == NMH - 1),
                        )

            # mm2(g) depends on the ACT/DVE relu^2 of mm1(g)'s psum, and the
            # first mm2 of a pass also waits on the previous pass's combine
            # (ys release); the PE is in-order, so emit mm2(g) two mm1
            # groups later to hide both latencies.
            pending = []  # [(hc, mh, kt, wt), ...]
            for hc in range(NHC):
                if (si, hc) not in w_chunks:
                    wt = wpool.tile([TILE, WCOLS], BF16, tag="w",
                                    name=f"w{si}_{hc}")
                    nc.sync.dma_start(wt[:], wr[si, hc])
                    w_chunks[(si, hc)] = wt
                wt = w_chunks[(si, hc)]
                if hc == 1:
                    # next pass's LayerNorm/conf: in-order ACT/DVE streams
                    # reach them mid-pass instead of after all relu ops
                    get_stage_a(pass_idx + 1)
                if hc == 2:
                    get_transposes(pass_idx + 1)
                for mh in range(NMH):
                    pk = pps.tile([TILE, ntok], F32, tag="pk")
                    for kc in range(NKC):
                        nc.tensor.matmul(
                            pk[:],
                            wt[:, kc * HCHUNK + mh * TILE : kc * HCHUNK + (mh + 1) * TILE],
                            hT[:, kc, :],
                            start=(kc == 0),
                            stop=(kc == NKC - 1),
                        )
                    kt = kpool.tile([TILE, ntok], BF16, tag="kt")
                    kr = kpool.tile([TILE, ntok], BF16, tag="kr")
                    if zero_bias:
                        # relu on DVE keeps the mm2-feeding path off the
                        # (LayerNorm-busy) ACT engine
                        nc.vector.tensor_scalar_max(kr[:], pk[:], 0.0)
                    else:
                        col = hc * NMH + mh
                        nc.scalar.activation(
                            kr[:], pk[:], AF.Relu,
                            bias=b1_sb[:, col : col + 1], scale=1.0,
                        )
                    nc.vector.tensor_mul(kt[:], kr[:], kr[:])
                    pending.append((hc, mh, kt, wt))
                    if hc == 0 and pending_stage_c:
                        # previous pass's combine, one tile per mm1 group:
                        # frees that pass's ys psum banks incrementally right
                        # behind this pass's first relus
                        pending_stage_c.pop(0)()
                    if len(pending) > 2:
                        emit_mm2(*pending.pop(0))
            while pending:
                emit_mm2(*pending.pop(0))

            def stage_c_tile(t, ys=ys, x_t=x_t, s_t=s_t, tile_off=tile_off):
                row0 = (tile_off + t) * TILE
                ot = opool.tile([TILE, C], F32, tag="o")
                for ncx in range(NC2):
                    nc.vector.scalar_tensor_tensor(
                        ot[:, ncx * 512 : (ncx + 1) * 512],
                        ys[t * NC2 + ncx][:],
                        s_t[t][:],
                        x_t[t][:, ncx * 512 : (ncx + 1) * 512],
                        op0=OP.mult,
                        op1=OP.add,
                    )
                nc.sync.dma_start(yc[row0 : row0 + TILE, :], ot[:])

            while pending_stage_c:
                pending_stage_c.pop(0)()
            pending_stage_c.extend(
                (lambda t=t: stage_c_tile(t)) for t in range(nt)
            )
        while pending_stage_c:
            pending_stage_c.pop(0)()

    _split_excess_waits(nc, 1)
    return nc


# ---------------------------------------------------------------------------
# Host-side dispatch
# ---------------------------------------------------------------------------


def _partitions(total, max_part, max_len):
    if total == 0:
        yield ()
        return
    if max_len == 0:
        return
    for first in range(min(total, max_part), 0, -1):
        for rest in _partitions(total - first, first, max_len - 1):
            yield (first,) + rest


def _try_pack(tiles, Tvec):
    """Greedy: assign each expert (desc) slot instances (8 per slot type).
    Returns assign list aligned with `tiles` order, or None."""
    avail = [list(range(NCORES)) for _ in Tvec]
    order_i = sorted(range(len(tiles)), key=lambda i: -tiles[i])
    assign = [None] * len(tiles)
    sizes = sorted(range(len(Tvec)), key=lambda j: -Tvec[j])
    for i in order_i:
        rem = tiles[i]
        inst = []
        while rem > 0:
            # largest slot type with size <= rem, else smallest type >= rem
            pick = None
            for j in sizes:
                if avail[j] and Tvec[j] <= rem:
                    pick = j
                    break
            if pick is None:
                for j in reversed(sizes):
                    if avail[j]:
                        pick = j
                        break
            if pick is None:
                return None
            c = avail[pick].pop(0)
            inst.append((pick, c))
            rem -= Tvec[pick]
        assign[i] = inst
    return assign


def _pack_slots(tiles):
    """Choose per-core slot sizes Tvec (identical structure on all cores)
    and an (expert -> slot instances) assignment minimizing per-core tiles."""
    total = sum(tiles)
    pmin = int(math.ceil(total / NCORES))
    for P in range(pmin, pmin + 4):
        cands = sorted(_partitions(P, P, 6), key=len)
        for Tvec in cands:
            a = _try_pack(tiles, list(Tvec))
            if a is not None:
                return list(Tvec), a
    # fallback: one dedicated slot per expert on every core
    Tvec = [int(math.ceil(t / NCORES)) for t in tiles]
    assign = [[(j, c) for c in range(NCORES)] for j in range(len(tiles))]
    return Tvec, assign


def _prepare(x, winners, gamma, beta, w1, w2, wc, bc):
    x = np.ascontiguousarray(np.asarray(x, dtype=np.float32))
    winners = np.asarray(winners).reshape(-1).astype(np.int64)
    gamma = np.asarray(gamma, dtype=np.float32)
    beta = np.asarray(beta, dtype=np.float32)
    w1 = np.asarray(w1, dtype=np.float32)
    w2 = np.asarray(w2, dtype=np.float32)
    wc = np.asarray(wc, dtype=np.float32)
    bc = np.asarray(bc, dtype=np.float32)

    B, T, C = x.shape
    E, _, H = w1.shape
    N = B * T
    xf = x.reshape(N, C)

    order = np.argsort(winners, kind="stable")
    counts = np.bincount(winners, minlength=E)

    present = [e for e in range(E) if counts[e] > 0]
    tiles_e = {e: int(math.ceil(counts[e] / TILE)) for e in present}

    Tvec, assign = _pack_slots([tiles_e[e] for e in present])
    # assign[i] = list of (slot_j, core_c) instances for present[i]
    S = len(Tvec)

    # slot_expert[c][j]: which expert's weights core c streams in slot j
    # (dummy instances reuse expert present[0]'s weights; their tokens are 0)
    slot_expert = [[present[0]] * S for _ in range(NCORES)]
    # token index list per (core, slot): length Tvec[j]*TILE, -1 = padding
    slot_idx = [
        [np.full(Tvec[j] * TILE, -1, dtype=np.int64) for j in range(S)]
        for c in range(NCORES)
    ]
    pos = 0
    for i, e in enumerate(present):
        n_e = int(counts[e])
        toks = order[pos : pos + n_e]
        pos += n_e
        filled = 0
        for (j, c) in assign[i]:
            slot_expert[c][j] = e
            cap = Tvec[j] * TILE
            take = min(cap, n_e - filled)
            if take > 0:
                slot_idx[c][j][:take] = toks[filled : filled + take]
                filled += take
        assert filled == n_e

    per_core_idx = [np.concatenate(slot_idx[c]) for c in range(NCORES)]
    M = per_core_idx[0].size

    passes = []
    tile_off = 0
    for j in range(S):
        k = 0
        while k < Tvec[j]:
            nt = min(2, Tvec[j] - k)
            passes.append((j, tile_off + k, nt))
            k += nt
        tile_off += Tvec[j]

    # fold gamma/beta; build per-EXPERT folded tensors once, then per-core
    # stacks indexed by that core's slot->expert table
    NKC = C // TILE
    NMH = HCHUNK // TILE
    NHC = H // HCHUNK
    zero_bias = bool(np.all(beta == 0.0))
    wrearr_e = {}
    wcb_e = {}
    bc_e = {}
    b1b_e = {}
    for e in present:
        w1f = (w1[e] * gamma[:, None]).astype(NP_BF16)
        w2f = w2[e].astype(NP_BF16)
        # re-layout into the exact SBUF tile order so each (slot, hchunk) is
        # ONE contiguous [128, WCOLS] DMA (16KB per partition row)
        w1part = (
            w1f.reshape(NKC, TILE, NHC, HCHUNK)
            .transpose(2, 1, 0, 3)
            .reshape(NHC, TILE, NKC * HCHUNK)
        )
        w2part = (
            w2f.reshape(NHC, NMH, TILE, C)
            .transpose(0, 2, 1, 3)
            .reshape(NHC, TILE, NMH * C)
        )
        wrearr_e[e] = np.ascontiguousarray(
            np.concatenate([w1part, w2part], axis=2)
        )
        wcf = (wc[e] * gamma).astype(NP_BF16)
        wcb_e[e] = np.ascontiguousarray(
            np.broadcast_to(wcf[None, :], (TILE, C))
        )
        bc_e[e] = np.full((TILE, 1), float(bc[e] + float(beta @ wc[e])),
                          dtype=np.float32)
        if not zero_bias:
            b1 = beta @ w1[e]
            b1b_e[e] = np.ascontiguousarray(
                b1.reshape(H // TILE, TILE).T
            ).astype(np.float32)

    in_maps = []
    for c in range(NCORES):
        idx = per_core_idx[c]
        xcrows = np.zeros((M, C), dtype=np.float32)
        valid = idx >= 0
        xcrows[valid] = xf[idx[valid]]
        sl = slot_expert[c]
        m = {
            "xc": xcrows,
            "wr": np.stack([wrearr_e[e] for e in sl]),
            "wcb": np.stack([wcb_e[e] for e in sl]),
            "bcs": np.stack([bc_e[e] for e in sl]),
        }
        if not zero_bias:
            m["b1b"] = np.stack([b1b_e[e] for e in sl])
        in_maps.append(m)

    meta = dict(
        B=B, T=T, C=C, H=H, N=N, M=M, S=S, passes=passes,
        zero_bias=zero_bias, per_core_idx=per_core_idx,
    )
    return in_maps, meta


def _assemble(results, meta):
    N, C = meta["N"], meta["C"]
    out = np.empty((N, C), dtype=np.float32)
    seen = np.zeros(N, dtype=bool)
    for c in range(NCORES):
        idx = meta["per_core_idx"][c]
        valid = idx >= 0
        out[idx[valid]] = results[c]["yc"][valid]
        seen[idx[valid]] = True
    assert seen.all()
    return out.reshape(meta["B"], meta["T"], C)


def kernel_with_results(x, winners, gamma, beta, w1, w2, wc, bc, **run_kwargs):
    in_maps, meta = _prepare(x, winners, gamma, beta, w1, w2, wc, bc)
    nc = _build_program(
        meta["C"], meta["H"], meta["M"], meta["S"], meta["passes"],
        meta["zero_bias"],
    )
    res = run_bass_kernel_spmd(nc, in_maps, core_ids=list(range(NCORES)), **run_kwargs)
    return _assemble(res.results, meta), res


def kernel(x, winners, gamma, beta, w1, w2, wc, bc):
    out, _ = kernel_with_results(x, winners, gamma, beta, w1, w2, wc, bc)
    return out


# revision 55
# speedup vs baseline: 1.0254x; 1.0071x over previous
"""CaMoE block (LayerNorm -> per-expert squared-ReLU FFN with top-1 routing,
confidence-scaled combine, residual) on 8 Trainium2 NeuronCores.

Strategy (token-parallel with expert-grouped tiles):
  * Host: stable-sort tokens by winning expert, pad each expert group to a
    multiple of 128*8 so every core receives the SAME number of 128-token
    tiles per expert. This makes the SPMD program identical across cores
    while every 128-token tile has a single expert.
  * Device (per core): for each 128-token tile: LayerNorm (token-major),
    confidence sigmoid(h.wc+bc) and straight-through scale c/(c+1e-6);
    transpose h via the PE; then stream the expert's W1/W2 in H-chunks and
    run  kT = relu(W1^T h^T)^2  (PE + DVE) and  y += kT^T W2chunk  (PE),
    finally  out = y*scale + x  (DVE) and DMA out.
  * Host: scatter rows back to their original token positions.

gamma/beta of the LayerNorm are folded into W1/wc on the host (plus an
additive H-bias when beta != 0), so the device computes the pre-affine LN.
All matmuls run in bf16 with fp32 PSUM accumulation.
"""

import math
import os
from contextlib import ExitStack

import numpy as np

import concourse.bass as bass
import concourse.mybir as mybir
import concourse.tile as tile
from concourse.bass_utils import run_bass_kernel_spmd
from concourse.masks import make_identity
from concourse.tile import TileContext, ScopedClock

AF = mybir.ActivationFunctionType
OP = mybir.AluOpType
BF16 = mybir.dt.bfloat16
F32 = mybir.dt.float32
NP_BF16 = mybir.dt.np(BF16)

NCORES = 8
TILE = 128
HCHUNK = 512
LN_EPS = 1e-5

# ---------------------------------------------------------------------------
# Workarounds for the walrus build in this environment: it encodes at most
# ONE semaphore wait per instruction and cannot split multi-wait
# instructions itself ("Too many sync wait commands"). We (a) emit the
# TileContext tail-drain waits one-per-NoOp and (b) post-process the whole
# program to hoist excess waits onto same-engine NoOps.
# ---------------------------------------------------------------------------


def _patched_drain_and_barrier(self, tick_clock, wait_clock):
    probe = self.nc.sync.nop(nofuse=True)
    wait_clock.add_sem_waits(probe.ins, ScopedClock({None: tick_clock.global_clock}))
    si = probe.ins.sync_info
    waits = list(si.on_wait) if si is not None and si.on_wait else []
    if len(waits) > 1:
        probe.ins.sync_info = mybir.SyncInfo(on_wait=[waits[0]], on_update=[])
        for w in waits[1:]:
            n = self.nc.sync.nop(nofuse=True)
            n.ins.sync_info = mybir.SyncInfo(on_wait=[w], on_update=[])
    self.nc.sync.drain()
    self.nc.all_engine_barrier()
    assert self.sems is not None
    popped = self.nc._tile_sem_poison_stack.pop()
    assert popped is self._sem_poison
    self.nc.clear_and_free_semaphores(list(self.sems.allocated().values()))
    self.nc.all_engine_barrier()


TileContext._drain_and_barrier = _patched_drain_and_barrier


def _split_excess_waits(nc, max_waits: int = 1):
    for fn in nc.m.functions:
        for bb in fn.blocks:
            insts = list(bb.instructions)
            out = []
            changed = False
            for inst in insts:
                si = inst.sync_info
                waits = list(si.on_wait) if si is not None and si.on_wait else []
                if len(waits) > max_waits:
                    extra = waits[:-max_waits]
                    keep = waits[-max_waits:]
                    for j, w in enumerate(extra):
                        nop = mybir.InstNoOp(
                            name=f"{inst.name}-wsplit{j}", ins=[], outs=[]
                        )
                        nop.engine = inst.engine
                        nop.sync_info = mybir.SyncInfo(on_wait=[w], on_update=[])
                        out.append(nop)
                    inst.sync_info = mybir.SyncInfo(
                        on_wait=keep,
                        on_update=list(si.on_update) if si.on_update else [],
                    )
                    changed = True
                out.append(inst)
            if changed:
                bb.instructions = out


# ---------------------------------------------------------------------------
# Device program
# ---------------------------------------------------------------------------


def _build_program(C, H, M, S, passes, zero_bias):
    """Emit the SPMD Bass program. `passes` is a list of
    (slot, tile_offset, n_tiles<=2); every core runs the same program on its
    own data."""
    NKC = C // TILE          # K-tiles over C (8)
    NMH = HCHUNK // TILE     # M-tiles per H-chunk (4)
    NHC = H // HCHUNK        # H-chunks (8)
    NC2 = C // 512           # output column chunks (2)
    HN = H // TILE           # bias columns (32)

    WCOLS = NKC * HCHUNK + NMH * C  # w1-part then w2-part, tile-contiguous

    nc = bass.Bass("TRN2", target_bir_lowering=False, debug=False)
    xc = nc.dram_tensor("xc", [M, C], F32, kind="ExternalInput").ap()
    wr = nc.dram_tensor("wr", [S, NHC, TILE, WCOLS], BF16, kind="ExternalInput").ap()
    wcb = nc.dram_tensor("wcb", [S, TILE, C], BF16, kind="ExternalInput").ap()
    bcs = nc.dram_tensor("bcs", [S, TILE, 1], F32, kind="ExternalInput").ap()
    if not zero_bias:
        b1b = nc.dram_tensor("b1b", [S, TILE, HN], F32, kind="ExternalInput").ap()
    yc = nc.dram_tensor("yc", [M, C], F32, kind="ExternalOutput").ap()

    with TileContext(nc) as tc, ExitStack() as ctx:
        cpool = ctx.enter_context(tc.tile_pool(name="const", bufs=1))
        ident = cpool.tile([TILE, TILE], BF16, tag="ident")
        make_identity(nc, ident[:])
        epsc = cpool.tile([TILE, 1], F32, tag="eps")
        nc.gpsimd.memset(epsc[:], LN_EPS)

        # weights stay RESIDENT for a whole slot (8 chunks x 2MB); the pool
        # rotation naturally overlaps the next slot's loads with the current
        # slot's last-pass reads
        wpool = ctx.enter_context(tc.tile_pool(name="w", bufs=8))
        spool = ctx.enter_context(tc.tile_pool(name="slot", bufs=2))
        xpool = ctx.enter_context(tc.tile_pool(name="x", bufs=4))
        hpool = ctx.enter_context(tc.tile_pool(name="h", bufs=4))
        prpool = ctx.enter_context(tc.tile_pool(name="pr", bufs=2))
        htpool = ctx.enter_context(tc.tile_pool(name="ht", bufs=2))
        kpool = ctx.enter_context(tc.tile_pool(name="kt", bufs=4))
        opool = ctx.enter_context(tc.tile_pool(name="o", bufs=3))
        stpool = ctx.enter_context(tc.tile_pool(name="st", bufs=8))
        sqpool = ctx.enter_context(tc.tile_pool(name="sq", bufs=1))
        pps = ctx.enter_context(tc.tile_pool(name="pk", bufs=2, space="PSUM"))
        ppy = ctx.enter_context(tc.tile_pool(name="py", bufs=4, space="PSUM"))
        ppt = ctx.enter_context(tc.tile_pool(name="ptr", bufs=2, space="PSUM"))

        slot_consts = {}
        w_chunks = {}

        def get_slot_consts(si):
            # NOTE: spool bufs must cover the number of distinct slots alive
            # at once (current + next pass's). Entries are invalidated by the
            # pool's slot reuse; with bufs=2 and passes grouped by slot this
            # holds.
            if si not in slot_consts:
                wcb_sb = spool.tile([TILE, C], BF16, tag="wcb", name=f"wcb{si}")
                nc.sync.dma_start(wcb_sb[:], wcb[si])
                bct = spool.tile([TILE, 1], F32, tag="bc", name=f"bc{si}")
                nc.sync.dma_start(bct[:], bcs[si])
                b1_sb = None
                if not zero_bias:
                    b1_sb = spool.tile([TILE, HN], F32, tag="b1", name=f"b1{si}")
                    nc.sync.dma_start(b1_sb[:], b1b[si])
                slot_consts[si] = (wcb_sb, bct, b1_sb)
            return slot_consts[si]

        def emit_stage_a(pass_idx):
            si, tile_off, nt = passes[pass_idx]
            ntok = TILE * nt
            wcb_sb, bct, _ = get_slot_consts(si)
            x_t = []
            s_t = []
            hT = htpool.tile([TILE, NKC, ntok], BF16, tag="hT",
                             name=f"hT{pass_idx}")
            # ops are interleaved across the pass's tiles so the serial
            # dep-chain latency (with a sem hop per tiny op) is paid once,
            # not once per tile
            st = lambda tag: [stpool.tile([TILE, 1], F32, tag=tag, name=f"{tag}{pass_idx}_{t}") for t in range(nt)]
            for t in range(nt):
                row0 = (tile_off + t) * TILE
                xt = xpool.tile([TILE, C], F32, tag="x", name=f"x{pass_idx}_{t}")
                x_t.append(xt)
                nc.sync.dma_start(xt[:], xc[row0 : row0 + TILE, :])
            nsum, negmu, ssq, std, rs, nmrs = (
                st("nsum"), st("negmu"), st("ssq"), st("std"), st("rs"),
                st("nmrs"),
            )
            cdot, conf, cpe, rc = st("cdot"), st("conf"), st("cpe"), st("rc")
            hts = []
            for t in range(nt):
                nc.vector.reduce_sum(
                    nsum[t][:], x_t[t][:], axis=mybir.AxisListType.X,
                    negate=True,
                )
            for t in range(nt):
                nc.scalar.mul(negmu[t][:], nsum[t][:], 1.0 / C)
            if zero_bias:
                # Fast path: transpose xc = x - mu (available right after the
                # mean); 1/std commutes through relu^2 and folds into the
                # stage-C per-token scalar:
                #   relu(rs*(xc@W1))^2 @ W2 = rs^2 * (relu(xc@W1)^2 @ W2)
                for t in range(nt):
                    ht_ = hpool.tile([TILE, C], BF16, tag="h",
                                     name=f"h{pass_idx}_{t}")
                    hts.append(ht_)
                    nc.scalar.activation(
                        ht_[:], x_t[t][:], AF.Identity, bias=negmu[t][:],
                        scale=1.0,
                    )
                for t in range(nt):
                    sq = sqpool.tile([TILE, C], F32, tag="sq")
                    nc.scalar.activation(
                        sq[:], x_t[t][:], AF.Square, bias=negmu[t][:],
                        scale=1.0, accum_out=ssq[t][:],
                    )
                for t in range(nt):
                    nc.scalar.activation(
                        std[t][:], ssq[t][:], AF.Sqrt, bias=epsc[:],
                        scale=1.0 / C,
                    )
                for t in range(nt):
                    nc.vector.reciprocal(rs[t][:], std[t][:])
                for t in range(nt):
                    prod = prpool.tile([TILE, C], BF16, tag="prod")
                    nc.vector.scalar_tensor_tensor(
                        prod[:], hts[t][:], 1.0, wcb_sb[:], op0=OP.mult,
                        op1=OP.mult, accum_out=cdot[t][:],
                    )
                for t in range(nt):
                    nc.scalar.activation(
                        conf[t][:], cdot[t][:], AF.Sigmoid, bias=bct[:],
                        scale=rs[t][:],
                    )
                for t in range(nt):
                    nc.vector.tensor_scalar_add(cpe[t][:], conf[t][:], 1e-6)
                for t in range(nt):
                    nc.vector.reciprocal(rc[t][:], cpe[t][:])
                rs2 = st("rs2")
                sc0 = st("sc0")
                for t in range(nt):
                    nc.vector.tensor_mul(rs2[t][:], rs[t][:], rs[t][:])
                for t in range(nt):
                    nc.vector.tensor_mul(sc0[t][:], conf[t][:], rc[t][:])
                for t in range(nt):
                    sc = stpool.tile([TILE, 1], F32, tag="sc",
                                     name=f"sc{pass_idx}_{t}")
                    nc.vector.tensor_mul(sc[:], sc0[t][:], rs2[t][:])
                    s_t.append(sc)
                return x_t, s_t, hT, hts
            for t in range(nt):
                sq = sqpool.tile([TILE, C], F32, tag="sq")
                nc.scalar.activation(
                    sq[:], x_t[t][:], AF.Square, bias=negmu[t][:], scale=1.0,
                    accum_out=ssq[t][:],
                )
            for t in range(nt):
                nc.scalar.activation(
                    std[t][:], ssq[t][:], AF.Sqrt, bias=epsc[:], scale=1.0 / C
                )
            for t in range(nt):
                nc.vector.reciprocal(rs[t][:], std[t][:])
            for t in range(nt):
                nc.vector.tensor_mul(nmrs[t][:], negmu[t][:], rs[t][:])
            for t in range(nt):
                ht_ = hpool.tile([TILE, C], BF16, tag="h", name=f"h{pass_idx}_{t}")
                hts.append(ht_)
                nc.scalar.activation(
                    ht_[:], x_t[t][:], AF.Identity, bias=nmrs[t][:],
                    scale=rs[t][:],
                )
            for t in range(nt):
                prod = prpool.tile([TILE, C], BF16, tag="prod")
                nc.vector.scalar_tensor_tensor(
                    prod[:], hts[t][:], 1.0, wcb_sb[:], op0=OP.mult,
                    op1=OP.mult, accum_out=cdot[t][:],
                )
            for t in range(nt):
                nc.scalar.activation(
                    conf[t][:], cdot[t][:], AF.Sigmoid, bias=bct[:], scale=1.0
                )
            for t in range(nt):
                nc.vector.tensor_scalar_add(cpe[t][:], conf[t][:], 1e-6)
            for t in range(nt):
                nc.vector.reciprocal(rc[t][:], cpe[t][:])
            for t in range(nt):
                sc = stpool.tile([TILE, 1], F32, tag="sc", name=f"sc{pass_idx}_{t}")
                nc.vector.tensor_mul(sc[:], conf[t][:], rc[t][:])
                s_t.append(sc)
            return x_t, s_t, hT, hts

        def emit_transposes(pass_idx):
            # phase 2: emitted well after phase 1 so the DVE copies never
            # head-of-line block the relu^2 stream waiting on the LN chain
            _, _, nt = passes[pass_idx]
            x_t, s_t, hT, hts = stage_a[pass_idx]
            for t in range(nt):
                for kc2 in range(NKC // 2):
                    kc = kc2 * 2
                    pt = ppt.tile([TILE, 2 * TILE], BF16, tag="ptr")
                    nc.tensor.transpose(
                        pt[:, 0:TILE],
                        hts[t][:, kc * TILE : (kc + 1) * TILE], ident[:],
                    )
                    nc.tensor.transpose(
                        pt[:, TILE : 2 * TILE],
                        hts[t][:, (kc + 1) * TILE : (kc + 2) * TILE], ident[:],
                    )
                    nc.vector.tensor_copy(
                        hT[:, kc : kc + 2, t * TILE : (t + 1) * TILE],
                        pt[:].rearrange("p (k c) -> p k c", k=2),
                    )

        stage_a = {}
        transposed = set()

        def get_stage_a(pass_idx):
            if pass_idx < len(passes) and pass_idx not in stage_a:
                stage_a[pass_idx] = emit_stage_a(pass_idx)
            return stage_a.get(pass_idx)

        def get_transposes(pass_idx):
            if pass_idx < len(passes) and pass_idx not in transposed:
                get_stage_a(pass_idx)
                transposed.add(pass_idx)
                emit_transposes(pass_idx)

        pending_stage_c = []
        for pass_idx, (si, tile_off, nt) in enumerate(passes):
            ntok = TILE * nt
            get_transposes(pass_idx)
            x_t, s_t, hT, _ = stage_a[pass_idx]
            _, _, b1_sb = get_slot_consts(si)

            ys = [
                ppy.tile([TILE, 512], F32, tag="py", name=f"ys{i}")
                for i in range(nt * NC2)
            ]

            def emit_mm2(hc, mh, kt, wt):
                w2base = NKC * HCHUNK + mh * C
                for t in range(nt):
                    for ncx in range(NC2):
                        nc.tensor.matmul(
                            ys[t * NC2 + ncx][:],
                            kt[:, t * TILE : (t + 1) * TILE],
                            wt[:, w2base + ncx * 512 : w2base + (ncx + 1) * 512],
                            start=(hc == 0 and mh == 0),
                            stop=(hc == NHC - 1 and mh == NMH - 1),
                        )

            # mm2(g) depends on the ACT/DVE relu^2 of mm1(g)'s psum, and the
            # first mm2 of a pass also waits on the previous pass's combine
            # (ys release); the PE is in-order, so emit mm2(g) two mm1
            # groups later to hide both latencies.
            pending = []  # [(hc, mh, kt, wt), ...]
            for hc in range(NHC):
                if (si, hc) not in w_chunks:
                    wt = wpool.tile([TILE, WCOLS], BF16, tag="w",
                                    name=f"w{si}_{hc}")
                    nc.sync.dma_start(wt[:], wr[si, hc])
                    w_chunks[(si, hc)] = wt
                wt = w_chunks[(si, hc)]
                if hc == 1:
                    # next pass's LayerNorm/conf: in-order ACT/DVE streams
                    # reach them mid-pass instead of after all relu ops
                    get_stage_a(pass_idx + 1)
                if hc == 4:
                    get_transposes(pass_idx + 1)
                for mh in range(NMH):
                    pk = pps.tile([TILE, ntok], F32, tag="pk")
                    for kc in range(NKC):
                        nc.tensor.matmul(
                            pk[:],
                            wt[:, kc * HCHUNK + mh * TILE : kc * HCHUNK + (mh + 1) * TILE],
                            hT[:, kc, :],
                            start=(kc == 0),
                            stop=(kc == NKC - 1),
                        )
                    kt = kpool.tile([TILE, ntok], BF16, tag="kt")
                    kr = kpool.tile([TILE, ntok], BF16, tag="kr")
                    if zero_bias:
                        # relu on DVE keeps the mm2-feeding path off the
                        # (LayerNorm-busy) ACT engine
                        nc.vector.tensor_scalar_max(kr[:], pk[:], 0.0)
                    else:
                        col = hc * NMH + mh
                        nc.scalar.activation(
                            kr[:], pk[:], AF.Relu,
                            bias=b1_sb[:, col : col + 1], scale=1.0,
                        )
                    nc.vector.tensor_mul(kt[:], kr[:], kr[:])
                    pending.append((hc, mh, kt, wt))
                    if hc == 0 and pending_stage_c:
                        # previous pass's combine, one tile per mm1 group:
                        # frees that pass's ys psum banks incrementally right
                        # behind this pass's first relus
                        pending_stage_c.pop(0)()
                    if len(pending) > 2:
                        emit_mm2(*pending.pop(0))
            while pending:
                emit_mm2(*pending.pop(0))

            def stage_c_tile(t, ys=ys, x_t=x_t, s_t=s_t, tile_off=tile_off):
                row0 = (tile_off + t) * TILE
                ot = opool.tile([TILE, C], F32, tag="o")
                for ncx in range(NC2):
                    nc.vector.scalar_tensor_tensor(
                        ot[:, ncx * 512 : (ncx + 1) * 512],
                        ys[t * NC2 + ncx][:],
                        s_t[t][:],
                        x_t[t][:, ncx * 512 : (ncx + 1) * 512],
                        op0=OP.mult,
                        op1=OP.add,
                    )
                nc.sync.dma_start(yc[row0 : row0 + TILE, :], ot[:])

            while pending_stage_c:
                pending_stage_c.pop(0)()
            pending_stage_c.extend(
                (lambda t=t: stage_c_tile(t)) for t in range(nt)
            )
        while pending_stage_c:
            pending_stage_c.pop(0)()

    _split_excess_waits(nc, 1)
    return nc


# ---------------------------------------------------------------------------
# Host-side dispatch
# ---------------------------------------------------------------------------


def _partitions(total, max_part, max_len):
    if total == 0:
        yield ()
        return
    if max_len == 0:
        return
    for first in range(min(total, max_part), 0, -1):
        for rest in _partitions(total - first, first, max_len - 1):
            yield (first,) + rest


def _try_pack(tiles, Tvec):
    """Greedy: assign each expert (desc) slot instances (8 per slot type).
    Returns assign list aligned with `tiles` order, or None."""
    avail = [list(range(NCORES)) for _ in Tvec]
    order_i = sorted(range(len(tiles)), key=lambda i: -tiles[i])
    assign = [None] * len(tiles)
    sizes = sorted(range(len(Tvec)), key=lambda j: -Tvec[j])
    for i in order_i:
        rem = tiles[i]
        inst = []
        while rem > 0:
            # largest slot type with size <= rem, else smallest type >= rem
            pick = None
            for j in sizes:
                if avail[j] and Tvec[j] <= rem:
                    pick = j
                    break
            if pick is None:
                for j in reversed(sizes):
                    if avail[j]:
                        pick = j
                        break
            if pick is None:
                return None
            c = avail[pick].pop(0)
            inst.append((pick, c))
            rem -= Tvec[pick]
        assign[i] = inst
    return assign


def _pack_slots(tiles):
    """Choose per-core slot sizes Tvec (identical structure on all cores)
    and an (expert -> slot instances) assignment minimizing per-core tiles."""
    total = sum(tiles)
    pmin = int(math.ceil(total / NCORES))
    for P in range(pmin, pmin + 4):
        cands = sorted(_partitions(P, P, 6), key=len)
        for Tvec in cands:
            a = _try_pack(tiles, list(Tvec))
            if a is not None:
                return list(Tvec), a
    # fallback: one dedicated slot per expert on every core
    Tvec = [int(math.ceil(t / NCORES)) for t in tiles]
    assign = [[(j, c) for c in range(NCORES)] for j in range(len(tiles))]
    return Tvec, assign


def _prepare(x, winners, gamma, beta, w1, w2, wc, bc):
    x = np.ascontiguousarray(np.asarray(x, dtype=np.float32))
    winners = np.asarray(winners).reshape(-1).astype(np.int64)
    gamma = np.asarray(gamma, dtype=np.float32)
    beta = np.asarray(beta, dtype=np.float32)
    w1 = np.asarray(w1, dtype=np.float32)
    w2 = np.asarray(w2, dtype=np.float32)
    wc = np.asarray(wc, dtype=np.float32)
    bc = np.asarray(bc, dtype=np.float32)

    B, T, C = x.shape
    E, _, H = w1.shape
    N = B * T
    xf = x.reshape(N, C)

    order = np.argsort(winners, kind="stable")
    counts = np.bincount(winners, minlength=E)

    present = [e for e in range(E) if counts[e] > 0]
    tiles_e = {e: int(math.ceil(counts[e] / TILE)) for e in present}

    Tvec, assign = _pack_slots([tiles_e[e] for e in present])
    # assign[i] = list of (slot_j, core_c) instances for present[i]
    S = len(Tvec)

    # slot_expert[c][j]: which expert's weights core c streams in slot j
    # (dummy instances reuse expert present[0]'s weights; their tokens are 0)
    slot_expert = [[present[0]] * S for _ in range(NCORES)]
    # token index list per (core, slot): length Tvec[j]*TILE, -1 = padding
    slot_idx = [
        [np.full(Tvec[j] * TILE, -1, dtype=np.int64) for j in range(S)]
        for c in range(NCORES)
    ]
    pos = 0
    for i, e in enumerate(present):
        n_e = int(counts[e])
        toks = order[pos : pos + n_e]
        pos += n_e
        filled = 0
        for (j, c) in assign[i]:
            slot_expert[c][j] = e
            cap = Tvec[j] * TILE
            take = min(cap, n_e - filled)
            if take > 0:
                slot_idx[c][j][:take] = toks[filled : filled + take]
                filled += take
        assert filled == n_e

    per_core_idx = [np.concatenate(slot_idx[c]) for c in range(NCORES)]
    M = per_core_idx[0].size

    passes = []
    tile_off = 0
    for j in range(S):
        k = 0
        while k < Tvec[j]:
            nt = min(2, Tvec[j] - k)
            passes.append((j, tile_off + k, nt))
            k += nt
        tile_off += Tvec[j]

    # fold gamma/beta; build per-EXPERT folded tensors once, then per-core
    # stacks indexed by that core's slot->expert table
    NKC = C // TILE
    NMH = HCHUNK // TILE
    NHC = H // HCHUNK
    zero_bias = bool(np.all(beta == 0.0))
    wrearr_e = {}
    wcb_e = {}
    bc_e = {}
    b1b_e = {}
    for e in present:
        w1f = (w1[e] * gamma[:, None]).astype(NP_BF16)
        w2f = w2[e].astype(NP_BF16)
        # re-layout into the exact SBUF tile order so each (slot, hchunk) is
        # ONE contiguous [128, WCOLS] DMA (16KB per partition row)
        w1part = (
            w1f.reshape(NKC, TILE, NHC, HCHUNK)
            .transpose(2, 1, 0, 3)
            .reshape(NHC, TILE, NKC * HCHUNK)
        )
        w2part = (
            w2f.reshape(NHC, NMH, TILE, C)
            .transpose(0, 2, 1, 3)
            .reshape(NHC, TILE, NMH * C)
        )
        wrearr_e[e] = np.ascontiguousarray(
            np.concatenate([w1part, w2part], axis=2)
        )
        wcf = (wc[e] * gamma).astype(NP_BF16)
        wcb_e[e] = np.ascontiguousarray(
            np.broadcast_to(wcf[None, :], (TILE, C))
        )
        bc_e[e] = np.full((TILE, 1), float(bc[e] + float(beta @ wc[e])),
                          dtype=np.float32)
        if not zero_bias:
            b1 = beta @ w1[e]
            b1b_e[e] = np.ascontiguousarray(
                b1.reshape(H // TILE, TILE).T
            ).astype(np.float32)

    in_maps = []
    for c in range(NCORES):
        idx = per_core_idx[c]
        xcrows = np.zeros((M, C), dtype=np.float32)
        valid = idx >= 0
        xcrows[valid] = xf[idx[valid]]
        sl = slot_expert[c]
        m = {
            "xc": xcrows,
            "wr": np.stack([wrearr_e[e] for e in sl]),
            "wcb": np.stack([wcb_e[e] for e in sl]),
            "bcs": np.stack([bc_e[e] for e in sl]),
        }
        if not zero_bias:
            m["b1b"] = np.stack([b1b_e[e] for e in sl])
        in_maps.append(m)

    meta = dict(
        B=B, T=T, C=C, H=H, N=N, M=M, S=S, passes=passes,
        zero_bias=zero_bias, per_core_idx=per_core_idx,
    )
    return in_maps, meta


def _assemble(results, meta):
    N, C = meta["N"], meta["C"]
    out = np.empty((N, C), dtype=np.float32)
    seen = np.zeros(N, dtype=bool)
    for c in range(NCORES):
        idx = meta["per_core_idx"][c]
        valid = idx >= 0
        out[idx[valid]] = results[c]["yc"][valid]
        seen[idx[valid]] = True
    assert seen.all()
    return out.reshape(meta["B"], meta["T"], C)


def kernel_with_results(x, winners, gamma, beta, w1, w2, wc, bc, **run_kwargs):
    in_maps, meta = _prepare(x, winners, gamma, beta, w1, w2, wc, bc)
    nc = _build_program(
        meta["C"], meta["H"], meta["M"], meta["S"], meta["passes"],
        meta["zero_bias"],
    )
    res = run_bass_kernel_spmd(nc, in_maps, core_ids=list(range(NCORES)), **run_kwargs)
    return _assemble(res.results, meta), res


def kernel(x, winners, gamma, beta, w1, w2, wc, bc):
    out, _ = kernel_with_results(x, winners, gamma, beta, w1, w2, wc, bc)
    return out
